# revision 16
# baseline (speedup 1.0000x reference)
"""Trainium2 Bass kernel for DeepMultiBasisBilinearNet.

Strategy: pure data-parallel over the batch (16384/8 = 2048 rows per core).
All activations kept in transposed [D, B] layout on-device so that every
matmul contraction dim lands on SBUF partitions with zero on-device
transposes (host pre-transposes x and all weights). Matmuls run in bf16
(fp32 PSUM accumulation) except block-2's eigen projection, which only
feeds block-2's LayerNorm statistics and therefore tolerates fp8:
it runs as fp8e4 DoubleRow matmuls at 2x PE rate. The actual output path
never sees that fp8 tensor: the final head is computed exactly as
  out = a2 .* (wf_g2 @ h1 + (wf_g2 @ We2) @ inter2 + wf_g2 @ be2)
        + c2 (x) sf + tf
with Wfe2 = wf_g2 @ We2 ([10, 4096]) folded host-side, so block-2's
1024-wide eigen output is never materialized in bf16 at all.

LN statistics use DVE running sums across the 8 d-chunks (one [1,NB]
partition-reduction matvec per statistic instead of eight), keeping the
PE instruction stream almost entirely 512-cycle payload matmuls. The r/l
projection accumulation chains are interleaved across three PSUM banks.
Block-1's stats rows and block-2's stats+head rows share a single PSUM
bank (their live ranges are disjoint: block-1's row chain completes
before block-2's eigen starts, and the deferred head application runs
early in the next tile's r/l stream, before that tile's block-1 eigen).
"""

import sys

if "/opt/trn_rl_repo" not in sys.path:
    sys.path.insert(0, "/opt/trn_rl_repo")

import ml_dtypes
import numpy as np

import concourse.bass as bass
import concourse.tile as tile
from concourse import bacc, mybir
from concourse.bass_utils import run_bass_kernel_spmd

BF = mybir.dt.bfloat16
F32 = mybir.dt.float32
E4 = mybir.dt.float8e4
AF = mybir.ActivationFunctionType
ALU = mybir.AluOpType
PM = mybir.MatmulPerfMode

P = 128
B, D, H, R, OUT = 16384, 1024, 4, 1024, 10
HR = H * R                 # 4096
NCORES = 8
BC = B // NCORES           # 2048 rows per core
NB = 512                   # batch tile (matmul free dim, one PSUM bank)
DC = D // P                # 8 chunks of the model dim
JC = HR // P               # 32 chunks of the bilinear dim
KP = JC // 2               # 16 fp8 DoubleRow k-pairs
EGRP = 2                   # eigen-projection dout groups (PSUM pressure)
EGS = DC // EGRP           # douts per group
LN_EPS = 1e-5
WS_E2 = 1024.0             # we2 fp8 pre-scale (host); dequant on eviction


def _emit_rl(nc, pools, dram, blk, acts, fillers=None, make_fp8=False):
    """Right/left projections + interaction for one batch tile.

    fillers: optional list of zero-arg callbacks, sprinkled one per jc
    chunk into the matmul stream (used for deferred stats/apply whose
    inputs become ready while this stream keeps the PE busy).
    make_fp8: additionally emit the interaction as fp8e4 k-pair tiles
    [P, 2*NB] for the DoubleRow eigen (block 2).
    """
    wp, ip, i8p, pp, psr, cst = (
        pools["wp"], pools["ip"], pools["i8p"], pools["pp"], pools["ps_rl"],
        pools["const"],
    )
    br_sb = cst[f"br{blk}"]
    bl_sb = cst[f"bl{blk}"]
    fillers = list(fillers or [])

    inter = []
    it8s = []
    cur8 = None
    for jc in range(JC):
        wr_t = wp.tile([P, D], BF, tag="wrl")
        nc.sync.dma_start(out=wr_t[:], in_=dram[f"wr{blk}"][jc])
        wl_t = wp.tile([P, D], BF, tag="wrl")
        nc.sync.dma_start(out=wl_t[:], in_=dram[f"wl{blk}"][jc])

        # r and l chains INTERLEAVED: consecutive matmuls alternate PSUM
        # banks, hiding the PE drain + weight-load serialization that a
        # same-bank accumulation chain incurs.
        ps_r = psr.tile([P, NB], F32, tag="rl")
        ps_l = psr.tile([P, NB], F32, tag="rl")
        for dc in range(DC):
            nc.tensor.matmul(
                ps_r[:], wr_t[:, dc * P:(dc + 1) * P], acts[dc][:],
                start=(dc == 0), stop=(dc == DC - 1),
            )
            nc.tensor.matmul(
                ps_l[:], wl_t[:, dc * P:(dc + 1) * P], acts[dc][:],
                start=(dc == 0), stop=(dc == DC - 1),
            )
        if fillers:
            fillers.pop(0)()
        # evict right off PSUM immediately (ACT) so the bank frees fast
        tmp_r = pp.tile([P, NB], BF, tag="tmp_r")
        nc.scalar.activation(tmp_r[:], ps_r[:], AF.Identity,
                             bias=br_sb[:, jc:jc + 1])
        tmp_l = pp.tile([P, NB], BF, tag="tmp_l")
        nc.scalar.activation(tmp_l[:], ps_l[:], AF.Identity,
                             bias=bl_sb[:, jc:jc + 1])

        it = ip.tile([P, NB], BF, tag="inter", bufs=32)
        nc.vector.tensor_mul(it[:], tmp_r[:], tmp_l[:])
        inter.append(it)
        if make_fp8:
            if jc % 2 == 0:
                cur8 = i8p.tile([P, 2 * NB], E4, tag="it8", bufs=16)
                it8s.append(cur8)
            # fp8 copy for the DoubleRow eigen (second DVE mul, fp8 out)
            nc.vector.tensor_mul(cur8[:, (jc % 2) * NB:(jc % 2 + 1) * NB],
                                 tmp_r[:], tmp_l[:])
    for f in fillers:
        f()
    return inter, it8s


def _emit_row_chain(nc, pools, st_mu, st_sq, tag):
    """rstd / -mu*rstd row pair from the PSUM mean/sq-mean rows.

    rstd = sqrt(1/(var+eps)): the reciprocal runs on DVE right after the
    var ops (no engine hop) and the Sqrt writes the bf16 row directly.
    """
    sb, cst = pools["sb"], pools["const"]
    mu = sb.tile([1, NB], F32, tag=f"mu{tag}", bufs=2)
    nc.scalar.copy(mu[:], st_mu)
    var = sb.tile([1, NB], F32, tag=f"var{tag}", bufs=2)
    nc.vector.scalar_tensor_tensor(var[:], mu[:], -1.0, st_mu,
                                   op0=ALU.mult, op1=ALU.mult)
    var2 = sb.tile([1, NB], F32, tag=f"va{tag}", bufs=2)
    nc.vector.scalar_tensor_tensor(var2[:], var[:], LN_EPS, st_sq,
                                   op0=ALU.add, op1=ALU.add)
    rv = sb.tile([1, NB], F32, tag=f"rv{tag}", bufs=2)
    nc.vector.reciprocal_approx_fast(out=rv[:], in_=var2[:])
    row = sb.tile([1, 2 * NB], BF, tag=f"row{tag}", bufs=3)
    nc.scalar.activation(row[:, 0:NB], rv[:], AF.Sqrt)
    nc.vector.scalar_tensor_tensor(row[:, NB:2 * NB], mu[:], -1.0,
                                   row[:, 0:NB], op0=ALU.mult, op1=ALU.mult)
    return row


def _emit_eigen(nc, pools, dram, blk, inter, acts):
    """Eigen projection + residual + LN statistics for one batch tile
    (block 1, bf16). Stats accumulate on DVE running sums; the tail
    (two matvecs + row chain) is emitted inline.
    """
    sb, wep_p, pse, pst, cst = (
        pools["sb"], pools["wep"], pools["ps_e"], pools["ps_st"],
        pools["const"],
    )
    be_sb = cst[f"be{blk}"]
    inv_d = cst["inv_d"]       # [128, 1] bf16 filled with 1/D
    st = pst.tile([64, NB], F32, tag="st")
    mss = [sb.tile([P, NB], BF, tag=f"ms1{g}", bufs=1, name=f"ms1{g}")
           for g in range(EGRP - 1)]
    qss = [sb.tile([P, NB], BF, tag=f"qs1{g}", bufs=1, name=f"qs1{g}")
           for g in range(EGRP - 1)]
    hpre = []
    last_sqs = []

    for g in range(EGRP):
        if g < EGRP - 1:
            ms, qs = mss[g], qss[g]
        ps_es = [pse.tile([P, NB], F32, tag="eig", name=f"eig{i}")
                 for i in range(EGS)]
        for jc in range(JC):
            we_t = wep_p.tile([P, EGS * P], BF, tag="wep")
            nc.sync.dma_start(out=we_t[:], in_=dram[f"we{blk}"][g, jc])
            for di in range(EGS):
                nc.tensor.matmul(
                    ps_es[di][:], we_t[:, di * P:(di + 1) * P], inter[jc][:],
                    start=(jc == 0), stop=(jc == JC - 1),
                )
        # evictions first (unblock hpre consumers), running sums after;
        # one sum pair per group so the last group's chain is short
        for di in range(EGS):
            do = g * EGS + di
            hp = sb.tile([P, NB], BF, tag="hpre", bufs=10)
            nc.vector.scalar_tensor_tensor(hp[:], ps_es[di][:],
                                           be_sb[:, do:do + 1], acts[do][:],
                                           op0=ALU.add, op1=ALU.add)
            hpre.append(hp)
        if g < EGRP - 1:
            for di in range(EGS):
                do = g * EGS + di
                hp = hpre[do]
                if di == 0:
                    nc.vector.scalar_tensor_tensor(ms[:], hp[:], 0.0, hp[:],
                                                   op0=ALU.mult, op1=ALU.add)
                    nc.scalar.activation(qs[:], hp[:], AF.Square)
                else:
                    nc.vector.tensor_add(ms[:], ms[:], hp[:])
                    sq = sb.tile([P, NB], BF, tag="sq", bufs=2)
                    nc.scalar.activation(sq[:], hp[:], AF.Square)
                    nc.vector.tensor_add(qs[:], qs[:], sq[:])
        else:
            # last group: no serial add chain; squares only, the stats
            # matvecs accumulate the raw hp/sq tiles directly
            for di in range(EGS):
                do = g * EGS + di
                sq = sb.tile([P, NB], BF, tag="sqd", bufs=5,
                             name=f"sqd{di}")
                nc.scalar.activation(sq[:], hpre[do][:], AF.Square)
                last_sqs.append(sq)

    box = {}

    def t_mean():
        srcs = [mss[g] for g in range(EGRP - 1)] + hpre[-EGS:]
        for i, s in enumerate(srcs):
            nc.tensor.matmul(st[0:1, :], inv_d[:, 0:1], s[:],
                             start=(i == 0), stop=(i == len(srcs) - 1))

    def t_sq():
        srcs = [qss[g] for g in range(EGRP - 1)] + last_sqs
        for i, s in enumerate(srcs):
            nc.tensor.matmul(st[32:33, :], inv_d[:, 0:1], s[:],
                             start=(i == 0), stop=(i == len(srcs) - 1))

    def t_rowbcast():
        row = _emit_row_chain(nc, pools, st[0:1, :], st[32:33, :], "")
        box["a"], box["c"] = _emit_ln_bcast_bf(nc, pools, row)

    return hpre, box, [t_mean, t_sq, t_rowbcast]



def _emit_rl2_commute(nc, pools, dram, hpre1, box, fillers,
                      late_fillers=None, late_start=8):
    """Block-2 r/l projections consuming the pre-LN residual hpre1 with
    block-1's LN row scalars applied on the (DVE) eviction path:
      r2 = a1 .* psum + c1 .* v~r + kr2
    v~r = (wr2 diag(g1)) @ 1 and kr2 = wr2 @ b1 + br2 are host-folded.
    fillers: block-1's deferred stats matvecs + row/bcast chain, emitted
    between jc=0's matmuls and its eviction so the PE stream never waits
    on them (the scheduler slots them in when their inputs land).
    """
    wp, ip, i8p, pp, psr, cst = (
        pools["wp"], pools["ip"], pools["i8p"], pools["pp"], pools["ps_rl"],
        pools["const"],
    )
    vr_sb, vl_sb = cst["vr2"], cst["vl2"]
    kr_sb, kl_sb = cst["kr2"], cst["kl2"]

    inter = []
    it8s = []
    cur8 = None
    raw0 = None
    for jc in range(JC):
        wr_t = wp.tile([P, D], BF, tag="wrl")
        nc.sync.dma_start(out=wr_t[:], in_=dram["wr2"][jc])
        wl_t = wp.tile([P, D], BF, tag="wrl")
        nc.sync.dma_start(out=wl_t[:], in_=dram["wl2"][jc])

        ps_r = psr.tile([P, NB], F32, tag="rl")
        ps_l = psr.tile([P, NB], F32, tag="rl")
        for dc in range(DC):
            nc.tensor.matmul(
                ps_r[:], wr_t[:, dc * P:(dc + 1) * P], hpre1[dc][:],
                start=(dc == 0), stop=(dc == DC - 1),
            )
            nc.tensor.matmul(
                ps_l[:], wl_t[:, dc * P:(dc + 1) * P], hpre1[dc][:],
                start=(dc == 0), stop=(dc == DC - 1),
            )
        if jc == 0:
            # jc0: raw ACT eviction (frees the banks without needing the
            # a/c broadcasts); its LN fixup chain is emitted at jc1, after
            # the block-1 stats fillers have produced the broadcasts
            fillers.pop(0)()
            raw_r = pp.tile([P, NB], BF, tag="u")
            nc.scalar.copy(raw_r[:], ps_r[:])
            raw_l = pp.tile([P, NB], BF, tag="w")
            nc.scalar.copy(raw_l[:], ps_l[:])
            raw0 = (raw_r, raw_l)
            continue
        if jc == 1:
            for f in fillers:
                f()
            fillers = []
        if late_fillers and jc >= late_start:
            late_fillers.pop(0)()
        a_bf, c_bf = box["a"], box["c"]
        if raw0 is not None:
            src_r, src_l = raw0
            _emit_rl2_evict(nc, pools, src_r[:], src_l[:], 0, a_bf, c_bf,
                            inter, it8s)
            raw0 = None
        _emit_rl2_evict(nc, pools, ps_r[:], ps_l[:], jc, a_bf, c_bf,
                        inter, it8s)
    return inter, it8s


def _emit_rl2_evict(nc, pools, src_r, src_l, jc, a_bf, c_bf, inter, it8s):
    """LN-fixup eviction + interaction for one jc chunk of block-2's r/l.
    src_r/src_l may be PSUM banks or raw bf16 SBUF tiles."""
    ip, i8p, pp, cst = (pools["ip"], pools["i8p"], pools["pp"],
                        pools["const"])
    vr_sb, vl_sb = cst["vr2"], cst["vl2"]
    kr_sb, kl_sb = cst["kr2"], cst["kl2"]
    u_r = pp.tile([P, NB], BF, tag="u")
    nc.vector.tensor_mul(u_r[:], src_r, a_bf[:])
    tmp_r = pp.tile([P, NB], BF, tag="tmp_r")
    nc.vector.scalar_tensor_tensor(tmp_r[:], c_bf[:], vr_sb[:, jc:jc + 1],
                                   u_r[:], op0=ALU.mult, op1=ALU.add)
    u_l = pp.tile([P, NB], BF, tag="w")
    nc.vector.tensor_mul(u_l[:], src_l, a_bf[:])
    tmp_l = pp.tile([P, NB], BF, tag="tmp_l")
    nc.vector.scalar_tensor_tensor(tmp_l[:], c_bf[:], vl_sb[:, jc:jc + 1],
                                   u_l[:], op0=ALU.mult, op1=ALU.add)
    tmp_lk = pp.tile([P, NB], BF, tag="tmp_e2")
    nc.scalar.activation(tmp_lk[:], tmp_l[:], AF.Identity,
                         bias=kl_sb[:, jc:jc + 1])

    # inter = (tmp_r + kr) * (tmp_l + kl); fp8 copy first (the
    # DoubleRow eigen consumes it sooner than the head reads bf16)
    if jc % 2 == 0:
        cur8 = i8p.tile([P, 2 * NB], E4, tag="it8", bufs=16,
                        name=f"it8_{jc}")
        it8s.append(cur8)
    cur8 = it8s[-1]
    nc.vector.scalar_tensor_tensor(
        cur8[:, (jc % 2) * NB:(jc % 2 + 1) * NB], tmp_r[:],
        kr_sb[:, jc:jc + 1], tmp_lk[:], op0=ALU.add, op1=ALU.mult)
    it = ip.tile([P, NB], BF, tag="inter", bufs=32, name=f"it_{jc}")
    nc.vector.scalar_tensor_tensor(it[:], tmp_r[:], kr_sb[:, jc:jc + 1],
                                   tmp_lk[:], op0=ALU.add, op1=ALU.mult)
    inter.append(it)


def _emit_eigen2(nc, pools, dram, inter, it8s, h1, final=False):
    """Block-2 eigen in fp8 DoubleRow (feeds LN stats only) + exact head
    accumulation.

    st bank rows: 0:OUT head accumulator, 32:33 mean, 64:65 sq-mean.
    Returns (st, row_box, tail); tail = two stats matvecs + row chain,
    deferred into the next tile's r/l stream (or interleaved into the
    head matmul stream when final=True).
    """
    sb, wep_p, pp, pse, pst, cst = (
        pools["sb"], pools["wep"], pools["pp"], pools["ps_e"],
        pools["ps_st"], pools["const"],
    )
    be_sb = cst["be2"]
    inv_d = cst["inv_d"]
    st = pst.tile([P, NB], F32, tag="st")
    mss = [sb.tile([P, NB], BF, tag=f"ms2{g}", bufs=1, name=f"ms2{g}")
           for g in range(EGRP)]
    qss = [sb.tile([P, NB], BF, tag=f"qs2{g}", bufs=1, name=f"qs2{g}")
           for g in range(EGRP)]

    for g in range(EGRP):
        ms, qs = mss[g], qss[g]
        ps_es = [pse.tile([P, NB], F32, tag="eig", name=f"e2g{i}")
                 for i in range(EGS)]
        for kp in range(KP):
            we_t = wep_p.tile([P, EGS * 2 * P], E4, tag="wep8", bufs=8)
            nc.sync.dma_start(out=we_t[:], in_=dram["we2q8"][g, kp])
            rhs = it8s[kp][:].rearrange("p (two n) -> p two n", two=2)
            for di in range(EGS):
                lhsT = we_t[:, di * 2 * P:(di + 1) * 2 * P].rearrange(
                    "p (two m) -> p two m", two=2)
                nc.tensor.matmul(ps_es[di][:], lhsT, rhs,
                                 start=(kp == 0), stop=(kp == KP - 1),
                                 perf_mode=PM.DoubleRow)
        for di in range(EGS):
            do = g * EGS + di
            # dequant eviction: hp = (psum/WS_E2 + be2) + h1
            t8 = pp.tile([P, NB], BF, tag="tmp_e2")
            nc.scalar.activation(t8[:], ps_es[di][:], AF.Identity,
                                 bias=be_sb[:, do:do + 1], scale=1.0 / WS_E2)
            hp = pp.tile([P, NB], BF, tag="hp2")
            nc.vector.tensor_add(hp[:], t8[:], h1[do][:])
            if di == 0:
                nc.vector.scalar_tensor_tensor(ms[:], hp[:], 0.0, hp[:],
                                               op0=ALU.mult, op1=ALU.add)
                nc.scalar.activation(qs[:], hp[:], AF.Square)
            else:
                nc.vector.tensor_add(ms[:], ms[:], hp[:])
                sq = pp.tile([P, NB], BF, tag="sq2")
                nc.scalar.activation(sq[:], hp[:], AF.Square)
                nc.vector.tensor_add(qs[:], qs[:], sq[:])

    row_box = {}

    def t_mean():
        for g in range(EGRP):
            nc.tensor.matmul(st[32:33, :], inv_d[:, 0:1], mss[g][:],
                             start=(g == 0), stop=(g == EGRP - 1))

    def t_sq():
        for g in range(EGRP):
            nc.tensor.matmul(st[64:65, :], inv_d[:, 0:1], qss[g][:],
                             start=(g == 0), stop=(g == EGRP - 1))

    def t_row():
        row_box["row"] = _emit_row_chain(nc, pools, st[32:33, :],
                                         st[64:65, :], "2")

    tail = [t_mean, t_sq, t_row]

    # exact head accumulation into rows 0:OUT of the same bank:
    #   hd = wf_g2 @ h1 + Wfe2 @ inter2   (all inputs long-ready)
    for dc in range(DC):
        nc.tensor.matmul(st[0:OUT, :], cst["wf"][:, dc * OUT:(dc + 1) * OUT],
                         h1[dc][:], start=(dc == 0), stop=False)
    for jc in range(JC):
        if final and tail and jc in (2, 4, 6):
            tail.pop(0)()
        nc.tensor.matmul(st[0:OUT, :], cst["wfe"][:, jc * OUT:(jc + 1) * OUT],
                         inter[jc][:], start=False, stop=(jc == JC - 1))
    if final:
        for f in tail:
            f()
        tail = []
    return st, row_box, tail


def _emit_ln_bcast_bf(nc, pools, row):
    """Broadcast [a | c] across partitions (two K=1 bf16 matmuls) and
    evict to bf16 SBUF immediately so the PSUM banks free early and the
    apply runs at bf16 DVE rate."""
    sb, pse, cst = pools["sb"], pools["ps_e"], pools["const"]
    ones_r = cst["ones_r"]
    a_ps = pse.tile([P, NB], F32, tag="eig", name="a_b")
    nc.tensor.matmul(a_ps[:], ones_r[:, :], row[:, 0:NB], start=True,
                     stop=True)
    c_ps = pse.tile([P, NB], F32, tag="eig", name="c_b")
    nc.tensor.matmul(c_ps[:], ones_r[:, :], row[:, NB:2 * NB],
                     start=True, stop=True)
    a_bf = sb.tile([P, NB], BF, tag="abf", bufs=2)
    nc.scalar.copy(a_bf[:], a_ps[:])
    c_bf = sb.tile([P, NB], BF, tag="cbf", bufs=2)
    nc.scalar.copy(c_bf[:], c_ps[:])
    return a_bf, c_bf


def _make_ln_apply(nc, pools, blk, hpre, box, outs):
    """Per-chunk LN-apply closures (2 DVE + 1 ACT each), interleaved into
    block-2's r/l stream so h1 is materialized while the PE streams."""
    sb, pp, cst = pools["sb"], pools["pp"], pools["const"]
    g_sb = cst[f"g{blk}"]
    bb_sb = cst[f"bb{blk}"]

    def one(do):
        def emit():
            u = pp.tile([P, NB], BF, tag="u")
            nc.vector.tensor_mul(u[:], hpre[do][:], box["a"][:])
            w = pp.tile([P, NB], BF, tag="w")
            nc.vector.tensor_add(w[:], u[:], box["c"][:])
            ho = sb.tile([P, NB], BF, tag=f"h{blk}", bufs=10)
            nc.scalar.activation(ho[:], w[:], AF.Identity,
                                 bias=bb_sb[:, do:do + 1],
                                 scale=g_sb[:, do:do + 1])
            outs.append(ho)
        return emit

    return [one(do) for do in range(DC)]


def build_program(bc=BC):
    """Build the per-core SPMD program. bc = rows per core."""
    nt = bc // NB
    nc = bacc.Bacc("TRN2", target_bir_lowering=False)

    dram = {
        "xT": nc.dram_tensor("xT", [D, bc], BF, kind="ExternalInput"),
        # wf is pre-folded with the block-2 LN gain g2 (host side)
        "wf": nc.dram_tensor("wf", [P, DC * OUT], BF, kind="ExternalInput"),
        # wfe = (wf_g2 @ we2) panels, contraction over the bilinear dim
        "wfe": nc.dram_tensor("wfe", [P, JC * OUT], BF, kind="ExternalInput"),
        "sf": nc.dram_tensor("sf", [OUT, 1], F32, kind="ExternalInput"),
        "tf": nc.dram_tensor("tf", [OUT, 1], F32, kind="ExternalInput"),
        "behead": nc.dram_tensor("behead", [OUT, 1], F32,
                                 kind="ExternalInput"),
        "outT": nc.dram_tensor("outT", [OUT, bc], F32, kind="ExternalOutput"),
        # block-2 eigen weights, fp8e4, pre-scaled by WS_E2, DoubleRow
        # k-pair layout [g, kp, p_j, (di, two, p_d)]
        "we2q8": nc.dram_tensor("we2q8", [EGRP, KP, P, EGS * 2 * P], E4,
                                kind="ExternalInput"),
    }
    for blk in (1, 2):
        dram[f"wr{blk}"] = nc.dram_tensor(f"wr{blk}", [JC, P, D], BF,
                                          kind="ExternalInput")
        dram[f"wl{blk}"] = nc.dram_tensor(f"wl{blk}", [JC, P, D], BF,
                                          kind="ExternalInput")
        dram[f"be{blk}"] = nc.dram_tensor(f"be{blk}", [P, DC], F32,
                                          kind="ExternalInput")
    for nm in ("br1", "bl1", "vr2", "vl2", "kr2", "kl2"):
        dram[nm] = nc.dram_tensor(nm, [P, JC], F32, kind="ExternalInput")
    for nm in ("g1", "bb1"):
        dram[nm] = nc.dram_tensor(nm, [P, DC], F32, kind="ExternalInput")
    if True:
        pass
    dram["we1"] = nc.dram_tensor("we1", [EGRP, JC, P, EGS * P], BF,
                                 kind="ExternalInput")

    with tile.TileContext(nc) as tc:
        with (
            tc.tile_pool(name="sb", bufs=2) as sb,
            tc.tile_pool(name="wp", bufs=6) as wp,
            tc.tile_pool(name="wep", bufs=12) as wep_p,
            tc.tile_pool(name="ip", bufs=32) as ip,
            tc.tile_pool(name="i8p", bufs=16) as i8p,
            tc.tile_pool(name="pp", bufs=3) as pp,
            tc.tile_pool(name="const", bufs=1) as cstp,
            tc.tile_pool(name="ps_rl", bufs=3, space="PSUM") as ps_rl,
            tc.tile_pool(name="ps_e", bufs=4, space="PSUM") as ps_e,
            tc.tile_pool(name="ps_st", bufs=1, space="PSUM") as ps_st,
        ):
            # warmup first: memset-fed throwaway matmuls start the PE before
            # any DMA lands, lifting the HAM clock gate to 8/8 early
            wm_l = cstp.tile([P, P], BF, tag="wm_l", name="wm_l")
            nc.vector.memset(wm_l[:], 0.0)
            wm_r = cstp.tile([P, NB], BF, tag="wm_r", name="wm_r")
            nc.vector.memset(wm_r[:], 0.0)
            for i in range(16):
                wps = ps_rl.tile([P, NB], F32, tag="rl", name=f"warm{i}")
                nc.tensor.matmul(wps[:], wm_l[:], wm_r[:],
                                 start=True, stop=True)

            cst = {}
            const_names = [("br1", JC), ("bl1", JC), ("vr2", JC),
                           ("vl2", JC), ("kr2", JC), ("kl2", JC),
                           ("be1", DC), ("be2", DC), ("g1", DC), ("bb1", DC)]
            for nm, cols in const_names:
                cst[nm] = cstp.tile([P, cols], F32, tag=nm, name=nm)
                nc.gpsimd.dma_start(out=cst[nm][:], in_=dram[nm][:])
            cst["inv_d"] = cstp.tile([P, 1], BF, tag="inv_d", name="inv_d")
            nc.vector.memset(cst["inv_d"][:], 1.0 / D)
            cst["ones_r"] = cstp.tile([1, P], BF, tag="ones_r", name="ones_r")
            nc.vector.memset(cst["ones_r"][:], 1.0)
            cst["eps"] = cstp.tile([1, 1], F32, tag="eps", name="eps")
            nc.vector.memset(cst["eps"][:], LN_EPS)
            cst["wf"] = cstp.tile([P, DC * OUT], BF, tag="wf", name="wf_sb")
            nc.gpsimd.dma_start(out=cst["wf"][:], in_=dram["wf"][:])
            cst["wfe"] = cstp.tile([P, JC * OUT], BF, tag="wfe", name="wfe_sb")
            nc.gpsimd.dma_start(out=cst["wfe"][:], in_=dram["wfe"][:])
            for nm in ("sf", "tf", "behead"):
                cst[nm] = cstp.tile([OUT, 1], F32, tag=nm, name=f"{nm}_sb")
                nc.gpsimd.dma_start(out=cst[nm][:], in_=dram[nm][:])

            pools = {
                "sb": sb, "wp": wp, "wep": wep_p, "ip": ip, "i8p": i8p,
                "pp": pp, "const": cst, "ps_rl": ps_rl, "ps_e": ps_e,
                "ps_st": ps_st,
            }
            ones_r = cst["ones_r"]

            def emit_head_apply(st, row, t):
                """out = a2 .* (hd + behead) + sf (x) c2 + tf, from the
                head accumulator in st rows 0:OUT."""
                a_ps = ps_e.tile([P, NB], F32, tag="eig", name="ha_b")
                nc.tensor.matmul(a_ps[0:OUT, :], ones_r[:, 0:OUT],
                                 row[:, 0:NB], start=True, stop=True)
                c_ps = ps_e.tile([P, NB], F32, tag="eig", name="hc_b")
                nc.tensor.matmul(c_ps[0:OUT, :], ones_r[:, 0:OUT],
                                 row[:, NB:2 * NB], start=True, stop=True)
                hd2 = sb.tile([OUT, NB], F32, tag="hd2", bufs=1)
                nc.scalar.activation(hd2[:], st[0:OUT, :], AF.Identity,
                                     bias=cst["behead"][:])
                a_sb = sb.tile([OUT, NB], F32, tag="hab", bufs=1)
                nc.scalar.copy(a_sb[:], a_ps[0:OUT, :])
                u = sb.tile([OUT, NB], F32, tag="hu", bufs=1)
                nc.vector.tensor_mul(u[:], hd2[:], a_sb[:])
                v = sb.tile([OUT, NB], F32, tag="hv", bufs=1)
                nc.vector.scalar_tensor_tensor(v[:], c_ps[0:OUT, :],
                                               cst["sf"][:], u[:],
                                               op0=ALU.mult, op1=ALU.add)
                out_sb = sb.tile([OUT, NB], F32, tag="osb", bufs=2)
                nc.scalar.activation(out_sb[:], v[:], AF.Identity,
                                     bias=cst["tf"][:])
                nc.gpsimd.dma_start(out=dram["outT"][:, t * NB:(t + 1) * NB],
                                    in_=out_sb[:])

            # pending = (st2, row_box, t, tail) for the tile whose block-2
            # stats matvecs + row chain + head application are deferred
            # into the next tile's block-1 r/l stream (fillers). Running
            # them early also frees the shared stats/head PSUM bank before
            # the next tile's block-1 eigen needs it.
            pending = None
            for t in range(nt):
                x_bf = []
                for dc in range(DC):
                    xt = sb.tile([P, NB], BF, tag="xbf", bufs=10)
                    xq = nc.scalar if t == 0 else nc.sync
                    xq.dma_start(
                        out=xt[:],
                        in_=dram["xT"][dc * P:(dc + 1) * P,
                                       t * NB:(t + 1) * NB],
                    )
                    x_bf.append(xt)

                if pending is not None:
                    st_prev, row2_box, t_prev, tail_prev = pending
                    prev_tail = list(tail_prev)
                    prev_tail.append(
                        lambda s=st_prev, rb=row2_box, tp=t_prev:
                        emit_head_apply(s, rb["row"], tp))
                else:
                    prev_tail = []
                inter1, _ = _emit_rl(nc, pools, dram, 1, x_bf,
                                     fillers=prev_tail)
                hpre1, box1, tail1 = _emit_eigen(nc, pools, dram, 1, inter1,
                                                 x_bf)
                h1 = []
                apply_fs = _make_ln_apply(nc, pools, 1, hpre1, box1, h1)
                final = (t == nt - 1)
                inter2, it8s = _emit_rl2_commute(nc, pools, dram, hpre1,
                                                 box1, tail1,
                                                 late_fillers=apply_fs,
                                                 late_start=2 if final
                                                 else 8)
                st2, row2_box, tail2 = _emit_eigen2(nc, pools, dram, inter2,
                                                    it8s, h1, final=final)
                pending = (st2, row2_box, t, tail2)

            # final tile: its tail was interleaved into the head stream
            st_prev, row2_box, t_prev, _ = pending
            emit_head_apply(st_prev, row2_box["row"], t_prev)
    nc.compile()
    return nc


def _bf(a):
    return np.ascontiguousarray(a.astype(ml_dtypes.bfloat16))


def prep_inputs(inputs, bc=BC, ncores=NCORES):
    """Host-side shard + transpose + bf16/fp8 conversion. Returns in_maps."""
    f = {k: np.asarray(v, dtype=np.float32) for k, v in inputs.items()}

    shared = {}
    for side in ("r", "l"):
        w = f[f"w{side}1"].reshape(HR, D)                  # [j, d]
        panel = w.reshape(JC, P, DC, P).transpose(0, 3, 2, 1)
        shared[f"w{side}1"] = _bf(panel.reshape(JC, P, D))
        shared[f"b{side}1"] = np.ascontiguousarray(
            f[f"b{side}1"].reshape(JC, P).T)                # [128, 32]
    # block-2 r/l: g1-folded panels + LN-commute fixup vectors
    g1_64 = f["g1"].astype(np.float64)
    b1_64 = f["b1"].astype(np.float64)
    for side in ("r", "l"):
        w64 = f[f"w{side}2"].reshape(HR, D).astype(np.float64)
        wg = w64 * g1_64[None, :]                           # W~ = W diag(g1)
        panel = wg.astype(np.float32).reshape(JC, P, DC, P).transpose(
            0, 3, 2, 1)
        shared[f"w{side}2"] = _bf(panel.reshape(JC, P, D))
        v = wg.sum(axis=1)                                  # v~ = W~ @ 1
        shared[f"v{side}2"] = np.ascontiguousarray(
            v.astype(np.float32).reshape(JC, P).T)          # [128, 32]
        k = w64 @ b1_64 + f[f"b{side}2"].reshape(HR).astype(np.float64)
        shared[f"k{side}2"] = np.ascontiguousarray(
            k.astype(np.float32).reshape(JC, P).T)          # [128, 32]
    for blk in (1, 2):
        shared[f"be{blk}"] = np.ascontiguousarray(
            f[f"be{blk}"].reshape(DC, P).T)                 # [128, 8]
    shared["g1"] = np.ascontiguousarray(f["g1"].reshape(DC, P).T)
    shared["bb1"] = np.ascontiguousarray(f["b1"].reshape(DC, P).T)

    # block-1 eigen: bf16 panels [g, jc, p_j, (di, p_d)]
    weT = f["we1"].T                                        # [j, d_out]
    panel = weT.reshape(JC, P, EGRP, EGS * P).transpose(2, 0, 1, 3)
    shared["we1"] = _bf(panel)                              # [g, jc, p, 512]

    # block-2 eigen: fp8e4 DoubleRow panels [g, kp, p_j, (di, two, p_d)],
    # pre-scaled so weight values sit in fp8's normal range
    weT2 = f["we2"].T                                       # [4096, 1024]
    pan8 = weT2.reshape(KP, 2, P, EGRP, EGS, P).transpose(3, 0, 2, 4, 1, 5)
    pan8 = np.clip(pan8 * WS_E2, -240.0, 240.0)
    shared["we2q8"] = np.ascontiguousarray(
        pan8.reshape(EGRP, KP, P, EGS * 2 * P).astype(ml_dtypes.float8_e4m3))

    # head folding (block-2 LN never applied as tensors):
    #   out = a2 .* (wf_g2 @ h1 + Wfe2 @ inter2 + behead) + sf (x) c2 + tf
    wf64 = f["wf"].astype(np.float64)
    g2_64 = f["g2"].astype(np.float64)
    we2_64 = f["we2"].astype(np.float64)
    wf_g2 = wf64 * g2_64[None, :]                           # [OUT, D]
    shared["wf"] = _bf(wf_g2.astype(np.float32).T.reshape(DC, P, OUT)
                       .transpose(1, 0, 2).reshape(P, DC * OUT))
    wfe2 = wf_g2 @ we2_64                                   # [OUT, HR]
    shared["wfe"] = _bf(wfe2.astype(np.float32).T.reshape(JC, P, OUT)
                        .transpose(1, 0, 2).reshape(P, JC * OUT))
    shared["behead"] = np.ascontiguousarray(
        (wf_g2 @ f["be2"].astype(np.float64)).reshape(OUT, 1)
        .astype(np.float32))
    shared["sf"] = np.ascontiguousarray(
        wf_g2.sum(axis=1).reshape(OUT, 1).astype(np.float32))
    shared["tf"] = np.ascontiguousarray(
        (wf64 @ f["b2"].astype(np.float64) + f["bf"]).reshape(OUT, 1)
        .astype(np.float32))

    x = f["x"]
    in_maps = []
    for c in range(ncores):
        m = dict(shared)
        m["xT"] = _bf(x[c * bc:(c + 1) * bc].T)             # [1024, bc]
        in_maps.append(m)
    return in_maps


_PROGRAM_CACHE = {}


def get_program(bc=BC):
    if bc not in _PROGRAM_CACHE:
        _PROGRAM_CACHE[bc] = build_program(bc)
    return _PROGRAM_CACHE[bc]


def kernel(**inputs):
    nc = get_program(BC)
    in_maps = prep_inputs(inputs, BC, NCORES)
    res = run_bass_kernel_spmd(nc, in_maps, core_ids=list(range(NCORES)))
    out = np.concatenate([res.results[c]["outT"] for c in range(NCORES)],
                         axis=1).T
    return np.ascontiguousarray(out.astype(np.float32))


if __name__ == "__main__":
    raise SystemExit("import kernel and call kernel(**inputs); see test.py")


# revision 17
# speedup vs baseline: 1.0059x; 1.0059x over previous
"""Trainium2 Bass kernel for DeepMultiBasisBilinearNet.

Strategy: pure data-parallel over the batch (16384/8 = 2048 rows per core).
All activations kept in transposed [D, B] layout on-device so that every
matmul contraction dim lands on SBUF partitions with zero on-device
transposes (host pre-transposes x and all weights). Matmuls run in bf16
(fp32 PSUM accumulation) except block-2's eigen projection, which only
feeds block-2's LayerNorm statistics and therefore tolerates fp8:
it runs as fp8e4 DoubleRow matmuls at 2x PE rate. The actual output path
never sees that fp8 tensor: the final head is computed exactly as
  out = a2 .* (wf_g2 @ h1 + (wf_g2 @ We2) @ inter2 + wf_g2 @ be2)
        + c2 (x) sf + tf
with Wfe2 = wf_g2 @ We2 ([10, 4096]) folded host-side, so block-2's
1024-wide eigen output is never materialized in bf16 at all.

LN statistics use DVE running sums across the 8 d-chunks (one [1,NB]
partition-reduction matvec per statistic instead of eight), keeping the
PE instruction stream almost entirely 512-cycle payload matmuls. The r/l
projection accumulation chains are interleaved across three PSUM banks.
Block-1's stats rows and block-2's stats+head rows share a single PSUM
bank (their live ranges are disjoint: block-1's row chain completes
before block-2's eigen starts, and the deferred head application runs
early in the next tile's r/l stream, before that tile's block-1 eigen).
"""

import sys

if "/opt/trn_rl_repo" not in sys.path:
    sys.path.insert(0, "/opt/trn_rl_repo")

import ml_dtypes
import numpy as np

import concourse.bass as bass
import concourse.tile as tile
from concourse import bacc, mybir
from concourse.bass_utils import run_bass_kernel_spmd

BF = mybir.dt.bfloat16
F32 = mybir.dt.float32
E4 = mybir.dt.float8e4
AF = mybir.ActivationFunctionType
ALU = mybir.AluOpType
PM = mybir.MatmulPerfMode

P = 128
B, D, H, R, OUT = 16384, 1024, 4, 1024, 10
HR = H * R                 # 4096
NCORES = 8
BC = B // NCORES           # 2048 rows per core
NB = 512                   # batch tile (matmul free dim, one PSUM bank)
DC = D // P                # 8 chunks of the model dim
JC = HR // P               # 32 chunks of the bilinear dim
KP = JC // 2               # 16 fp8 DoubleRow k-pairs
EGRP = 2                   # eigen-projection dout groups (PSUM pressure)
EGS = DC // EGRP           # douts per group
LN_EPS = 1e-5
WS_E2 = 1024.0             # we2 fp8 pre-scale (host); dequant on eviction


def _emit_rl(nc, pools, dram, blk, acts, fillers=None, make_fp8=False):
    """Right/left projections + interaction for one batch tile.

    fillers: optional list of zero-arg callbacks, sprinkled one per jc
    chunk into the matmul stream (used for deferred stats/apply whose
    inputs become ready while this stream keeps the PE busy).
    make_fp8: additionally emit the interaction as fp8e4 k-pair tiles
    [P, 2*NB] for the DoubleRow eigen (block 2).
    """
    wp, ip, i8p, pp, psr, cst = (
        pools["wp"], pools["ip"], pools["i8p"], pools["pp"], pools["ps_rl"],
        pools["const"],
    )
    br_sb = cst[f"br{blk}"]
    bl_sb = cst[f"bl{blk}"]
    fillers = list(fillers or [])

    inter = []
    it8s = []
    cur8 = None
    for jc in range(JC):
        wr_t = wp.tile([P, D], BF, tag="wrl")
        nc.sync.dma_start(out=wr_t[:], in_=dram[f"wr{blk}"][jc])
        wl_t = wp.tile([P, D], BF, tag="wrl")
        nc.sync.dma_start(out=wl_t[:], in_=dram[f"wl{blk}"][jc])

        # r and l chains INTERLEAVED: consecutive matmuls alternate PSUM
        # banks, hiding the PE drain + weight-load serialization that a
        # same-bank accumulation chain incurs.
        ps_r = psr.tile([P, NB], F32, tag="rl")
        ps_l = psr.tile([P, NB], F32, tag="rl")
        for dc in range(DC):
            nc.tensor.matmul(
                ps_r[:], wr_t[:, dc * P:(dc + 1) * P], acts[dc][:],
                start=(dc == 0), stop=(dc == DC - 1),
            )
            nc.tensor.matmul(
                ps_l[:], wl_t[:, dc * P:(dc + 1) * P], acts[dc][:],
                start=(dc == 0), stop=(dc == DC - 1),
            )
        if fillers:
            fillers.pop(0)()
        # evict right off PSUM immediately (ACT) so the bank frees fast
        tmp_r = pp.tile([P, NB], BF, tag="tmp_r")
        nc.scalar.activation(tmp_r[:], ps_r[:], AF.Identity,
                             bias=br_sb[:, jc:jc + 1])
        tmp_l = pp.tile([P, NB], BF, tag="tmp_l")
        nc.scalar.activation(tmp_l[:], ps_l[:], AF.Identity,
                             bias=bl_sb[:, jc:jc + 1])

        it = ip.tile([P, NB], BF, tag="inter", bufs=32)
        nc.vector.tensor_mul(it[:], tmp_r[:], tmp_l[:])
        inter.append(it)
        if make_fp8:
            if jc % 2 == 0:
                cur8 = i8p.tile([P, 2 * NB], E4, tag="it8", bufs=16)
                it8s.append(cur8)
            # fp8 copy for the DoubleRow eigen (second DVE mul, fp8 out)
            nc.vector.tensor_mul(cur8[:, (jc % 2) * NB:(jc % 2 + 1) * NB],
                                 tmp_r[:], tmp_l[:])
    for f in fillers:
        f()
    return inter, it8s


def _emit_row_chain(nc, pools, st_mu, st_sq, tag):
    """rstd / -mu*rstd row pair from the PSUM mean/sq-mean rows.

    rstd = sqrt(1/(var+eps)): the reciprocal runs on DVE right after the
    var ops (no engine hop) and the Sqrt writes the bf16 row directly.
    """
    sb, cst = pools["sb"], pools["const"]
    mu = sb.tile([1, NB], F32, tag=f"mu{tag}", bufs=2)
    nc.scalar.copy(mu[:], st_mu)
    var = sb.tile([1, NB], F32, tag=f"var{tag}", bufs=2)
    nc.vector.scalar_tensor_tensor(var[:], mu[:], -1.0, st_mu,
                                   op0=ALU.mult, op1=ALU.mult)
    var2 = sb.tile([1, NB], F32, tag=f"va{tag}", bufs=2)
    nc.vector.scalar_tensor_tensor(var2[:], var[:], LN_EPS, st_sq,
                                   op0=ALU.add, op1=ALU.add)
    rv = sb.tile([1, NB], F32, tag=f"rv{tag}", bufs=2)
    nc.vector.reciprocal_approx_fast(out=rv[:], in_=var2[:])
    row = sb.tile([1, 2 * NB], BF, tag=f"row{tag}", bufs=3)
    nc.scalar.activation(row[:, 0:NB], rv[:], AF.Sqrt)
    nc.vector.scalar_tensor_tensor(row[:, NB:2 * NB], mu[:], -1.0,
                                   row[:, 0:NB], op0=ALU.mult, op1=ALU.mult)
    return row


def _emit_eigen(nc, pools, dram, blk, inter, acts):
    """Eigen projection + residual + LN statistics for one batch tile
    (block 1, bf16). Stats accumulate on DVE running sums; the tail
    (two matvecs + row chain) is emitted inline.
    """
    sb, wep_p, pse, pst, cst = (
        pools["sb"], pools["wep"], pools["ps_e"], pools["ps_st"],
        pools["const"],
    )
    be_sb = cst[f"be{blk}"]
    inv_d = cst["inv_d"]       # [128, 1] bf16 filled with 1/D
    st = pst.tile([64, NB], F32, tag="st")
    mss = [sb.tile([P, NB], BF, tag=f"ms1{g}", bufs=1, name=f"ms1{g}")
           for g in range(EGRP - 1)]
    qss = [sb.tile([P, NB], BF, tag=f"qs1{g}", bufs=1, name=f"qs1{g}")
           for g in range(EGRP - 1)]
    hpre = []
    last_sqs = []

    for g in range(EGRP):
        if g < EGRP - 1:
            ms, qs = mss[g], qss[g]
        ps_es = [pse.tile([P, NB], F32, tag="eig", name=f"eig{i}")
                 for i in range(EGS)]
        for jc in range(JC):
            we_t = wep_p.tile([P, EGS * P], BF, tag="wep")
            nc.sync.dma_start(out=we_t[:], in_=dram[f"we{blk}"][g, jc])
            for di in range(EGS):
                nc.tensor.matmul(
                    ps_es[di][:], we_t[:, di * P:(di + 1) * P], inter[jc][:],
                    start=(jc == 0), stop=(jc == JC - 1),
                )
        # evictions first (unblock hpre consumers), running sums after;
        # one sum pair per group so the last group's chain is short
        for di in range(EGS):
            do = g * EGS + di
            hp = sb.tile([P, NB], BF, tag="hpre", bufs=10)
            nc.vector.scalar_tensor_tensor(hp[:], ps_es[di][:],
                                           be_sb[:, do:do + 1], acts[do][:],
                                           op0=ALU.add, op1=ALU.add)
            hpre.append(hp)
        if g < EGRP - 1:
            for di in range(EGS):
                do = g * EGS + di
                hp = hpre[do]
                if di == 0:
                    nc.vector.scalar_tensor_tensor(ms[:], hp[:], 0.0, hp[:],
                                                   op0=ALU.mult, op1=ALU.add)
                    nc.scalar.activation(qs[:], hp[:], AF.Square)
                else:
                    nc.vector.tensor_add(ms[:], ms[:], hp[:])
                    sq = sb.tile([P, NB], BF, tag="sq", bufs=2)
                    nc.scalar.activation(sq[:], hp[:], AF.Square)
                    nc.vector.tensor_add(qs[:], qs[:], sq[:])
        else:
            # last group: no serial add chain; squares only, the stats
            # matvecs accumulate the raw hp/sq tiles directly
            for di in range(EGS):
                do = g * EGS + di
                sq = sb.tile([P, NB], BF, tag="sqd", bufs=5,
                             name=f"sqd{di}")
                nc.scalar.activation(sq[:], hpre[do][:], AF.Square)
                last_sqs.append(sq)

    box = {}

    def t_mean():
        srcs = [mss[g] for g in range(EGRP - 1)] + hpre[-EGS:]
        for i, s in enumerate(srcs):
            nc.tensor.matmul(st[0:1, :], inv_d[:, 0:1], s[:],
                             start=(i == 0), stop=(i == len(srcs) - 1))

    def t_sq():
        srcs = [qss[g] for g in range(EGRP - 1)] + last_sqs
        for i, s in enumerate(srcs):
            nc.tensor.matmul(st[32:33, :], inv_d[:, 0:1], s[:],
                             start=(i == 0), stop=(i == len(srcs) - 1))

    def t_rowbcast():
        row = _emit_row_chain(nc, pools, st[0:1, :], st[32:33, :], "")
        box["a"], box["c"] = _emit_ln_bcast_bf(nc, pools, row)

    return hpre, box, [t_mean, t_sq, t_rowbcast]



def _emit_rl2_commute(nc, pools, dram, hpre1, box, fillers,
                      late_fillers=None, late_start=8):
    """Block-2 r/l projections consuming the pre-LN residual hpre1 with
    block-1's LN row scalars applied on the (DVE) eviction path:
      r2 = a1 .* psum + c1 .* v~r + kr2
    v~r = (wr2 diag(g1)) @ 1 and kr2 = wr2 @ b1 + br2 are host-folded.
    fillers: block-1's deferred stats matvecs + row/bcast chain, emitted
    between jc=0's matmuls and its eviction so the PE stream never waits
    on them (the scheduler slots them in when their inputs land).
    """
    wp, ip, i8p, pp, psr, cst = (
        pools["wp"], pools["ip"], pools["i8p"], pools["pp"], pools["ps_rl"],
        pools["const"],
    )
    vr_sb, vl_sb = cst["vr2"], cst["vl2"]
    kr_sb, kl_sb = cst["kr2"], cst["kl2"]

    inter = []
    it8s = []
    cur8 = None
    raw0 = None
    for jc in range(JC):
        wr_t = wp.tile([P, D], BF, tag="wrl")
        nc.sync.dma_start(out=wr_t[:], in_=dram["wr2"][jc])
        wl_t = wp.tile([P, D], BF, tag="wrl")
        nc.sync.dma_start(out=wl_t[:], in_=dram["wl2"][jc])

        ps_r = psr.tile([P, NB], F32, tag="rl")
        ps_l = psr.tile([P, NB], F32, tag="rl")
        for dc in range(DC):
            nc.tensor.matmul(
                ps_r[:], wr_t[:, dc * P:(dc + 1) * P], hpre1[dc][:],
                start=(dc == 0), stop=(dc == DC - 1),
            )
            nc.tensor.matmul(
                ps_l[:], wl_t[:, dc * P:(dc + 1) * P], hpre1[dc][:],
                start=(dc == 0), stop=(dc == DC - 1),
            )
        if jc == 0:
            for f in fillers:
                f()
        if late_fillers and jc >= late_start:
            late_fillers.pop(0)()
        a_bf, c_bf = box["a"], box["c"]
        _emit_rl2_evict(nc, pools, ps_r[:], ps_l[:], jc, a_bf, c_bf,
                        inter, it8s)
    return inter, it8s


def _emit_rl2_evict(nc, pools, src_r, src_l, jc, a_bf, c_bf, inter, it8s):
    """LN-fixup eviction + interaction for one jc chunk of block-2's r/l.
    src_r/src_l may be PSUM banks or raw bf16 SBUF tiles."""
    ip, i8p, pp, cst = (pools["ip"], pools["i8p"], pools["pp"],
                        pools["const"])
    vr_sb, vl_sb = cst["vr2"], cst["vl2"]
    kr_sb, kl_sb = cst["kr2"], cst["kl2"]
    u_r = pp.tile([P, NB], BF, tag="u")
    nc.vector.tensor_mul(u_r[:], src_r, a_bf[:])
    tmp_r = pp.tile([P, NB], BF, tag="tmp_r")
    nc.vector.scalar_tensor_tensor(tmp_r[:], c_bf[:], vr_sb[:, jc:jc + 1],
                                   u_r[:], op0=ALU.mult, op1=ALU.add)
    u_l = pp.tile([P, NB], BF, tag="w")
    nc.vector.tensor_mul(u_l[:], src_l, a_bf[:])
    tmp_l = pp.tile([P, NB], BF, tag="tmp_l")
    nc.vector.scalar_tensor_tensor(tmp_l[:], c_bf[:], vl_sb[:, jc:jc + 1],
                                   u_l[:], op0=ALU.mult, op1=ALU.add)
    tmp_lk = pp.tile([P, NB], BF, tag="tmp_e2")
    nc.scalar.activation(tmp_lk[:], tmp_l[:], AF.Identity,
                         bias=kl_sb[:, jc:jc + 1])

    # inter = (tmp_r + kr) * (tmp_l + kl); fp8 copy first (the
    # DoubleRow eigen consumes it sooner than the head reads bf16)
    if jc % 2 == 0:
        cur8 = i8p.tile([P, 2 * NB], E4, tag="it8", bufs=16,
                        name=f"it8_{jc}")
        it8s.append(cur8)
    cur8 = it8s[-1]
    nc.vector.scalar_tensor_tensor(
        cur8[:, (jc % 2) * NB:(jc % 2 + 1) * NB], tmp_r[:],
        kr_sb[:, jc:jc + 1], tmp_lk[:], op0=ALU.add, op1=ALU.mult)
    it = ip.tile([P, NB], BF, tag="inter", bufs=32, name=f"it_{jc}")
    nc.vector.scalar_tensor_tensor(it[:], tmp_r[:], kr_sb[:, jc:jc + 1],
                                   tmp_lk[:], op0=ALU.add, op1=ALU.mult)
    inter.append(it)


def _emit_eigen2(nc, pools, dram, inter, it8s, h1, final=False):
    """Block-2 eigen in fp8 DoubleRow (feeds LN stats only) + exact head
    accumulation.

    st bank rows: 0:OUT head accumulator, 32:33 mean, 64:65 sq-mean.
    Returns (st, row_box, tail); tail = two stats matvecs + row chain,
    deferred into the next tile's r/l stream (or interleaved into the
    head matmul stream when final=True).
    """
    sb, wep_p, pp, pse, pst, cst = (
        pools["sb"], pools["wep"], pools["pp"], pools["ps_e"],
        pools["ps_st"], pools["const"],
    )
    be_sb = cst["be2"]
    inv_d = cst["inv_d"]
    st = pst.tile([P, NB], F32, tag="st")
    mss = [sb.tile([P, NB], BF, tag=f"ms2{g}", bufs=1, name=f"ms2{g}")
           for g in range(EGRP)]
    qss = [sb.tile([P, NB], BF, tag=f"qs2{g}", bufs=1, name=f"qs2{g}")
           for g in range(EGRP)]

    for g in range(EGRP):
        ms, qs = mss[g], qss[g]
        ps_es = [pse.tile([P, NB], F32, tag="eig", name=f"e2g{i}")
                 for i in range(EGS)]
        for kp in range(KP):
            we_t = wep_p.tile([P, EGS * 2 * P], E4, tag="wep8", bufs=8)
            nc.sync.dma_start(out=we_t[:], in_=dram["we2q8"][g, kp])
            rhs = it8s[kp][:].rearrange("p (two n) -> p two n", two=2)
            for di in range(EGS):
                lhsT = we_t[:, di * 2 * P:(di + 1) * 2 * P].rearrange(
                    "p (two m) -> p two m", two=2)
                nc.tensor.matmul(ps_es[di][:], lhsT, rhs,
                                 start=(kp == 0), stop=(kp == KP - 1),
                                 perf_mode=PM.DoubleRow)
        for di in range(EGS):
            do = g * EGS + di
            # dequant eviction: hp = (psum/WS_E2 + be2) + h1
            t8 = pp.tile([P, NB], BF, tag="tmp_e2")
            nc.scalar.activation(t8[:], ps_es[di][:], AF.Identity,
                                 bias=be_sb[:, do:do + 1], scale=1.0 / WS_E2)
            hp = pp.tile([P, NB], BF, tag="hp2")
            nc.vector.tensor_add(hp[:], t8[:], h1[do][:])
            if di == 0:
                nc.vector.scalar_tensor_tensor(ms[:], hp[:], 0.0, hp[:],
                                               op0=ALU.mult, op1=ALU.add)
                nc.scalar.activation(qs[:], hp[:], AF.Square)
            else:
                nc.vector.tensor_add(ms[:], ms[:], hp[:])
                sq = pp.tile([P, NB], BF, tag="sq2")
                nc.scalar.activation(sq[:], hp[:], AF.Square)
                nc.vector.tensor_add(qs[:], qs[:], sq[:])

    row_box = {}

    def t_mean():
        for g in range(EGRP):
            nc.tensor.matmul(st[32:33, :], inv_d[:, 0:1], mss[g][:],
                             start=(g == 0), stop=(g == EGRP - 1))

    def t_sq():
        for g in range(EGRP):
            nc.tensor.matmul(st[64:65, :], inv_d[:, 0:1], qss[g][:],
                             start=(g == 0), stop=(g == EGRP - 1))

    def t_row():
        row_box["row"] = _emit_row_chain(nc, pools, st[32:33, :],
                                         st[64:65, :], "2")

    tail = [t_mean, t_sq, t_row]

    # exact head accumulation into rows 0:OUT of the same bank:
    #   hd = wf_g2 @ h1 + Wfe2 @ inter2   (all inputs long-ready)
    for dc in range(DC):
        nc.tensor.matmul(st[0:OUT, :], cst["wf"][:, dc * OUT:(dc + 1) * OUT],
                         h1[dc][:], start=(dc == 0), stop=False)
    for jc in range(JC):
        if final and tail and jc in (2, 4, 6):
            tail.pop(0)()
        nc.tensor.matmul(st[0:OUT, :], cst["wfe"][:, jc * OUT:(jc + 1) * OUT],
                         inter[jc][:], start=False, stop=(jc == JC - 1))
    if final:
        for f in tail:
            f()
        tail = []
    return st, row_box, tail


def _emit_ln_bcast_bf(nc, pools, row):
    """Broadcast [a | c] across partitions (two K=1 bf16 matmuls) and
    evict to bf16 SBUF immediately so the PSUM banks free early and the
    apply runs at bf16 DVE rate."""
    sb, pse, cst = pools["sb"], pools["ps_e"], pools["const"]
    ones_r = cst["ones_r"]
    a_ps = pse.tile([P, NB], F32, tag="eig", name="a_b")
    nc.tensor.matmul(a_ps[:], ones_r[:, :], row[:, 0:NB], start=True,
                     stop=True)
    c_ps = pse.tile([P, NB], F32, tag="eig", name="c_b")
    nc.tensor.matmul(c_ps[:], ones_r[:, :], row[:, NB:2 * NB],
                     start=True, stop=True)
    a_bf = sb.tile([P, NB], BF, tag="abf", bufs=2)
    nc.scalar.copy(a_bf[:], a_ps[:])
    c_bf = sb.tile([P, NB], BF, tag="cbf", bufs=2)
    nc.scalar.copy(c_bf[:], c_ps[:])
    return a_bf, c_bf


def _make_ln_apply(nc, pools, blk, hpre, box, outs):
    """Per-chunk LN-apply closures (2 DVE + 1 ACT each), interleaved into
    block-2's r/l stream so h1 is materialized while the PE streams."""
    sb, pp, cst = pools["sb"], pools["pp"], pools["const"]
    g_sb = cst[f"g{blk}"]
    bb_sb = cst[f"bb{blk}"]

    def one(do):
        def emit():
            u = pp.tile([P, NB], BF, tag="u")
            nc.vector.tensor_mul(u[:], hpre[do][:], box["a"][:])
            w = pp.tile([P, NB], BF, tag="w")
            nc.vector.tensor_add(w[:], u[:], box["c"][:])
            ho = sb.tile([P, NB], BF, tag=f"h{blk}", bufs=10)
            nc.scalar.activation(ho[:], w[:], AF.Identity,
                                 bias=bb_sb[:, do:do + 1],
                                 scale=g_sb[:, do:do + 1])
            outs.append(ho)
        return emit

    return [one(do) for do in range(DC)]


def build_program(bc=BC):
    """Build the per-core SPMD program. bc = rows per core."""
    nt = bc // NB
    nc = bacc.Bacc("TRN2", target_bir_lowering=False)

    dram = {
        "xT": nc.dram_tensor("xT", [D, bc], BF, kind="ExternalInput"),
        # wf is pre-folded with the block-2 LN gain g2 (host side)
        "wf": nc.dram_tensor("wf", [P, DC * OUT], BF, kind="ExternalInput"),
        # wfe = (wf_g2 @ we2) panels, contraction over the bilinear dim
        "wfe": nc.dram_tensor("wfe", [P, JC * OUT], BF, kind="ExternalInput"),
        "sf": nc.dram_tensor("sf", [OUT, 1], F32, kind="ExternalInput"),
        "tf": nc.dram_tensor("tf", [OUT, 1], F32, kind="ExternalInput"),
        "behead": nc.dram_tensor("behead", [OUT, 1], F32,
                                 kind="ExternalInput"),
        "outT": nc.dram_tensor("outT", [OUT, bc], F32, kind="ExternalOutput"),
        # block-2 eigen weights, fp8e4, pre-scaled by WS_E2, DoubleRow
        # k-pair layout [g, kp, p_j, (di, two, p_d)]
        "we2q8": nc.dram_tensor("we2q8", [EGRP, KP, P, EGS * 2 * P], E4,
                                kind="ExternalInput"),
    }
    for blk in (1, 2):
        dram[f"wr{blk}"] = nc.dram_tensor(f"wr{blk}", [JC, P, D], BF,
                                          kind="ExternalInput")
        dram[f"wl{blk}"] = nc.dram_tensor(f"wl{blk}", [JC, P, D], BF,
                                          kind="ExternalInput")
        dram[f"be{blk}"] = nc.dram_tensor(f"be{blk}", [P, DC], F32,
                                          kind="ExternalInput")
    for nm in ("br1", "bl1", "vr2", "vl2", "kr2", "kl2"):
        dram[nm] = nc.dram_tensor(nm, [P, JC], F32, kind="ExternalInput")
    for nm in ("g1", "bb1"):
        dram[nm] = nc.dram_tensor(nm, [P, DC], F32, kind="ExternalInput")
    if True:
        pass
    dram["we1"] = nc.dram_tensor("we1", [EGRP, JC, P, EGS * P], BF,
                                 kind="ExternalInput")

    with tile.TileContext(nc) as tc:
        with (
            tc.tile_pool(name="sb", bufs=2) as sb,
            tc.tile_pool(name="wp", bufs=6) as wp,
            tc.tile_pool(name="wep", bufs=12) as wep_p,
            tc.tile_pool(name="ip", bufs=32) as ip,
            tc.tile_pool(name="i8p", bufs=16) as i8p,
            tc.tile_pool(name="pp", bufs=3) as pp,
            tc.tile_pool(name="const", bufs=1) as cstp,
            tc.tile_pool(name="ps_rl", bufs=3, space="PSUM") as ps_rl,
            tc.tile_pool(name="ps_e", bufs=4, space="PSUM") as ps_e,
            tc.tile_pool(name="ps_st", bufs=1, space="PSUM") as ps_st,
        ):
            # warmup first: memset-fed throwaway matmuls start the PE before
            # any DMA lands, lifting the HAM clock gate to 8/8 early
            wm_l = cstp.tile([P, P], BF, tag="wm_l", name="wm_l")
            nc.vector.memset(wm_l[:], 0.0)
            wm_r = cstp.tile([P, NB], BF, tag="wm_r", name="wm_r")
            nc.vector.memset(wm_r[:], 0.0)
            for i in range(16):
                wps = ps_rl.tile([P, NB], F32, tag="rl", name=f"warm{i}")
                nc.tensor.matmul(wps[:], wm_l[:], wm_r[:],
                                 start=True, stop=True)

            cst = {}
            const_names = [("br1", JC), ("bl1", JC), ("vr2", JC),
                           ("vl2", JC), ("kr2", JC), ("kl2", JC),
                           ("be1", DC), ("be2", DC), ("g1", DC), ("bb1", DC)]
            for nm, cols in const_names:
                cst[nm] = cstp.tile([P, cols], F32, tag=nm, name=nm)
                nc.gpsimd.dma_start(out=cst[nm][:], in_=dram[nm][:])
            cst["inv_d"] = cstp.tile([P, 1], BF, tag="inv_d", name="inv_d")
            nc.vector.memset(cst["inv_d"][:], 1.0 / D)
            cst["ones_r"] = cstp.tile([1, P], BF, tag="ones_r", name="ones_r")
            nc.vector.memset(cst["ones_r"][:], 1.0)
            cst["eps"] = cstp.tile([1, 1], F32, tag="eps", name="eps")
            nc.vector.memset(cst["eps"][:], LN_EPS)
            cst["wf"] = cstp.tile([P, DC * OUT], BF, tag="wf", name="wf_sb")
            nc.gpsimd.dma_start(out=cst["wf"][:], in_=dram["wf"][:])
            cst["wfe"] = cstp.tile([P, JC * OUT], BF, tag="wfe", name="wfe_sb")
            nc.gpsimd.dma_start(out=cst["wfe"][:], in_=dram["wfe"][:])
            for nm in ("sf", "tf", "behead"):
                cst[nm] = cstp.tile([OUT, 1], F32, tag=nm, name=f"{nm}_sb")
                nc.gpsimd.dma_start(out=cst[nm][:], in_=dram[nm][:])

            pools = {
                "sb": sb, "wp": wp, "wep": wep_p, "ip": ip, "i8p": i8p,
                "pp": pp, "const": cst, "ps_rl": ps_rl, "ps_e": ps_e,
                "ps_st": ps_st,
            }
            ones_r = cst["ones_r"]

            def emit_head_apply(st, row, t):
                """out = a2 .* (hd + behead) + sf (x) c2 + tf, from the
                head accumulator in st rows 0:OUT."""
                a_ps = ps_e.tile([P, NB], F32, tag="eig", name="ha_b")
                nc.tensor.matmul(a_ps[0:OUT, :], ones_r[:, 0:OUT],
                                 row[:, 0:NB], start=True, stop=True)
                c_ps = ps_e.tile([P, NB], F32, tag="eig", name="hc_b")
                nc.tensor.matmul(c_ps[0:OUT, :], ones_r[:, 0:OUT],
                                 row[:, NB:2 * NB], start=True, stop=True)
                hd2 = sb.tile([OUT, NB], F32, tag="hd2", bufs=1)
                nc.scalar.activation(hd2[:], st[0:OUT, :], AF.Identity,
                                     bias=cst["behead"][:])
                a_sb = sb.tile([OUT, NB], F32, tag="hab", bufs=1)
                nc.scalar.copy(a_sb[:], a_ps[0:OUT, :])
                u = sb.tile([OUT, NB], F32, tag="hu", bufs=1)
                nc.vector.tensor_mul(u[:], hd2[:], a_sb[:])
                v = sb.tile([OUT, NB], F32, tag="hv", bufs=1)
                nc.vector.scalar_tensor_tensor(v[:], c_ps[0:OUT, :],
                                               cst["sf"][:], u[:],
                                               op0=ALU.mult, op1=ALU.add)
                out_sb = sb.tile([OUT, NB], F32, tag="osb", bufs=2)
                nc.scalar.activation(out_sb[:], v[:], AF.Identity,
                                     bias=cst["tf"][:])
                nc.gpsimd.dma_start(out=dram["outT"][:, t * NB:(t + 1) * NB],
                                    in_=out_sb[:])

            # pending = (st2, row_box, t, tail) for the tile whose block-2
            # stats matvecs + row chain + head application are deferred
            # into the next tile's block-1 r/l stream (fillers). Running
            # them early also frees the shared stats/head PSUM bank before
            # the next tile's block-1 eigen needs it.
            pending = None
            for t in range(nt):
                x_bf = []
                for dc in range(DC):
                    xt = sb.tile([P, NB], BF, tag="xbf", bufs=10)
                    xq = nc.scalar if t == 0 else nc.sync
                    xq.dma_start(
                        out=xt[:],
                        in_=dram["xT"][dc * P:(dc + 1) * P,
                                       t * NB:(t + 1) * NB],
                    )
                    x_bf.append(xt)

                if pending is not None:
                    st_prev, row2_box, t_prev, tail_prev = pending
                    prev_tail = list(tail_prev)
                    prev_tail.append(
                        lambda s=st_prev, rb=row2_box, tp=t_prev:
                        emit_head_apply(s, rb["row"], tp))
                else:
                    prev_tail = []
                inter1, _ = _emit_rl(nc, pools, dram, 1, x_bf,
                                     fillers=prev_tail)
                hpre1, box1, tail1 = _emit_eigen(nc, pools, dram, 1, inter1,
                                                 x_bf)
                h1 = []
                apply_fs = _make_ln_apply(nc, pools, 1, hpre1, box1, h1)
                final = (t == nt - 1)
                inter2, it8s = _emit_rl2_commute(nc, pools, dram, hpre1,
                                                 box1, tail1,
                                                 late_fillers=apply_fs,
                                                 late_start=2 if final
                                                 else 8)
                st2, row2_box, tail2 = _emit_eigen2(nc, pools, dram, inter2,
                                                    it8s, h1, final=final)
                pending = (st2, row2_box, t, tail2)

            # final tile: its tail was interleaved into the head stream
            st_prev, row2_box, t_prev, _ = pending
            emit_head_apply(st_prev, row2_box["row"], t_prev)
    nc.compile()
    return nc


def _bf(a):
    return np.ascontiguousarray(a.astype(ml_dtypes.bfloat16))


def prep_inputs(inputs, bc=BC, ncores=NCORES):
    """Host-side shard + transpose + bf16/fp8 conversion. Returns in_maps."""
    f = {k: np.asarray(v, dtype=np.float32) for k, v in inputs.items()}

    shared = {}
    for side in ("r", "l"):
        w = f[f"w{side}1"].reshape(HR, D)                  # [j, d]
        panel = w.reshape(JC, P, DC, P).transpose(0, 3, 2, 1)
        shared[f"w{side}1"] = _bf(panel.reshape(JC, P, D))
        shared[f"b{side}1"] = np.ascontiguousarray(
            f[f"b{side}1"].reshape(JC, P).T)                # [128, 32]
    # block-2 r/l: g1-folded panels + LN-commute fixup vectors
    g1_64 = f["g1"].astype(np.float64)
    b1_64 = f["b1"].astype(np.float64)
    for side in ("r", "l"):
        w64 = f[f"w{side}2"].reshape(HR, D).astype(np.float64)
        wg = w64 * g1_64[None, :]                           # W~ = W diag(g1)
        panel = wg.astype(np.float32).reshape(JC, P, DC, P).transpose(
            0, 3, 2, 1)
        shared[f"w{side}2"] = _bf(panel.reshape(JC, P, D))
        v = wg.sum(axis=1)                                  # v~ = W~ @ 1
        shared[f"v{side}2"] = np.ascontiguousarray(
            v.astype(np.float32).reshape(JC, P).T)          # [128, 32]
        k = w64 @ b1_64 + f[f"b{side}2"].reshape(HR).astype(np.float64)
        shared[f"k{side}2"] = np.ascontiguousarray(
            k.astype(np.float32).reshape(JC, P).T)          # [128, 32]
    for blk in (1, 2):
        shared[f"be{blk}"] = np.ascontiguousarray(
            f[f"be{blk}"].reshape(DC, P).T)                 # [128, 8]
    shared["g1"] = np.ascontiguousarray(f["g1"].reshape(DC, P).T)
    shared["bb1"] = np.ascontiguousarray(f["b1"].reshape(DC, P).T)

    # block-1 eigen: bf16 panels [g, jc, p_j, (di, p_d)]
    weT = f["we1"].T                                        # [j, d_out]
    panel = weT.reshape(JC, P, EGRP, EGS * P).transpose(2, 0, 1, 3)
    shared["we1"] = _bf(panel)                              # [g, jc, p, 512]

    # block-2 eigen: fp8e4 DoubleRow panels [g, kp, p_j, (di, two, p_d)],
    # pre-scaled so weight values sit in fp8's normal range
    weT2 = f["we2"].T                                       # [4096, 1024]
    pan8 = weT2.reshape(KP, 2, P, EGRP, EGS, P).transpose(3, 0, 2, 4, 1, 5)
    pan8 = np.clip(pan8 * WS_E2, -240.0, 240.0)
    shared["we2q8"] = np.ascontiguousarray(
        pan8.reshape(EGRP, KP, P, EGS * 2 * P).astype(ml_dtypes.float8_e4m3))

    # head folding (block-2 LN never applied as tensors):
    #   out = a2 .* (wf_g2 @ h1 + Wfe2 @ inter2 + behead) + sf (x) c2 + tf
    wf64 = f["wf"].astype(np.float64)
    g2_64 = f["g2"].astype(np.float64)
    we2_64 = f["we2"].astype(np.float64)
    wf_g2 = wf64 * g2_64[None, :]                           # [OUT, D]
    shared["wf"] = _bf(wf_g2.astype(np.float32).T.reshape(DC, P, OUT)
                       .transpose(1, 0, 2).reshape(P, DC * OUT))
    wfe2 = wf_g2 @ we2_64                                   # [OUT, HR]
    shared["wfe"] = _bf(wfe2.astype(np.float32).T.reshape(JC, P, OUT)
                        .transpose(1, 0, 2).reshape(P, JC * OUT))
    shared["behead"] = np.ascontiguousarray(
        (wf_g2 @ f["be2"].astype(np.float64)).reshape(OUT, 1)
        .astype(np.float32))
    shared["sf"] = np.ascontiguousarray(
        wf_g2.sum(axis=1).reshape(OUT, 1).astype(np.float32))
    shared["tf"] = np.ascontiguousarray(
        (wf64 @ f["b2"].astype(np.float64) + f["bf"]).reshape(OUT, 1)
        .astype(np.float32))

    x = f["x"]
    in_maps = []
    for c in range(ncores):
        m = dict(shared)
        m["xT"] = _bf(x[c * bc:(c + 1) * bc].T)             # [1024, bc]
        in_maps.append(m)
    return in_maps


_PROGRAM_CACHE = {}


def get_program(bc=BC):
    if bc not in _PROGRAM_CACHE:
        _PROGRAM_CACHE[bc] = build_program(bc)
    return _PROGRAM_CACHE[bc]


def kernel(**inputs):
    nc = get_program(BC)
    in_maps = prep_inputs(inputs, BC, NCORES)
    res = run_bass_kernel_spmd(nc, in_maps, core_ids=list(range(NCORES)))
    out = np.concatenate([res.results[c]["outT"] for c in range(NCORES)],
                         axis=1).T
    return np.ascontiguousarray(out.astype(np.float32))


if __name__ == "__main__":
    raise SystemExit("import kernel and call kernel(**inputs); see test.py")


# revision 20
# speedup vs baseline: 1.0082x; 1.0023x over previous
"""Trainium2 Bass kernel for DeepMultiBasisBilinearNet.

Strategy: pure data-parallel over the batch (16384/8 = 2048 rows per core).
All activations kept in transposed [D, B] layout on-device so that every
matmul contraction dim lands on SBUF partitions with zero on-device
transposes (host pre-transposes x and all weights). Matmuls run in bf16
(fp32 PSUM accumulation) except block-2's eigen projection, which only
feeds block-2's LayerNorm statistics and therefore tolerates fp8:
it runs as fp8e4 DoubleRow matmuls at 2x PE rate. The actual output path
never sees that fp8 tensor: the final head is computed exactly as
  out = a2 .* (wf_g2 @ h1 + (wf_g2 @ We2) @ inter2 + wf_g2 @ be2)
        + c2 (x) sf + tf
with Wfe2 = wf_g2 @ We2 ([10, 4096]) folded host-side, so block-2's
1024-wide eigen output is never materialized in bf16 at all.

LN statistics use DVE running sums across the 8 d-chunks (one [1,NB]
partition-reduction matvec per statistic instead of eight), keeping the
PE instruction stream almost entirely 512-cycle payload matmuls. The r/l
projection accumulation chains are interleaved across three PSUM banks.
Block-1's stats rows and block-2's stats+head rows share a single PSUM
bank (their live ranges are disjoint: block-1's row chain completes
before block-2's eigen starts, and the deferred head application runs
early in the next tile's r/l stream, before that tile's block-1 eigen).
"""

import sys

if "/opt/trn_rl_repo" not in sys.path:
    sys.path.insert(0, "/opt/trn_rl_repo")

import ml_dtypes
import numpy as np

import concourse.bass as bass
import concourse.tile as tile
from concourse import bacc, mybir
from concourse.bass_utils import run_bass_kernel_spmd

BF = mybir.dt.bfloat16
F32 = mybir.dt.float32
E4 = mybir.dt.float8e4
AF = mybir.ActivationFunctionType
ALU = mybir.AluOpType
PM = mybir.MatmulPerfMode

P = 128
B, D, H, R, OUT = 16384, 1024, 4, 1024, 10
HR = H * R                 # 4096
NCORES = 8
BC = B // NCORES           # 2048 rows per core
NB = 512                   # batch tile (matmul free dim, one PSUM bank)
DC = D // P                # 8 chunks of the model dim
JC = HR // P               # 32 chunks of the bilinear dim
KP = JC // 2               # 16 fp8 DoubleRow k-pairs
EGRP = 2                   # eigen-projection dout groups (PSUM pressure)
EGS = DC // EGRP           # douts per group
LN_EPS = 1e-5
WS_E2 = 1024.0             # we2 fp8 pre-scale (host); dequant on eviction


def _emit_rl(nc, pools, dram, blk, acts, fillers=None, make_fp8=False):
    """Right/left projections + interaction for one batch tile.

    fillers: optional list of zero-arg callbacks, sprinkled one per jc
    chunk into the matmul stream (used for deferred stats/apply whose
    inputs become ready while this stream keeps the PE busy).
    make_fp8: additionally emit the interaction as fp8e4 k-pair tiles
    [P, 2*NB] for the DoubleRow eigen (block 2).
    """
    wp, ip, i8p, pp, psr, cst = (
        pools["wp"], pools["ip"], pools["i8p"], pools["pp"], pools["ps_rl"],
        pools["const"],
    )
    br_sb = cst[f"br{blk}"]
    bl_sb = cst[f"bl{blk}"]
    fillers = list(fillers or [])

    inter = []
    it8s = []
    cur8 = None
    for jc in range(JC):
        wr_t = wp.tile([P, D], BF, tag="wrl")
        nc.sync.dma_start(out=wr_t[:], in_=dram[f"wr{blk}"][jc])
        wl_t = wp.tile([P, D], BF, tag="wrl")
        nc.sync.dma_start(out=wl_t[:], in_=dram[f"wl{blk}"][jc])

        # r and l chains INTERLEAVED: consecutive matmuls alternate PSUM
        # banks, hiding the PE drain + weight-load serialization that a
        # same-bank accumulation chain incurs.
        ps_r = psr.tile([P, NB], F32, tag="rl")
        ps_l = psr.tile([P, NB], F32, tag="rl")
        for dc in range(DC):
            nc.tensor.matmul(
                ps_r[:], wr_t[:, dc * P:(dc + 1) * P], acts[dc][:],
                start=(dc == 0), stop=(dc == DC - 1),
            )
            nc.tensor.matmul(
                ps_l[:], wl_t[:, dc * P:(dc + 1) * P], acts[dc][:],
                start=(dc == 0), stop=(dc == DC - 1),
            )
        if fillers:
            fillers.pop(0)()
        # evict right off PSUM immediately (ACT) so the bank frees fast
        tmp_r = pp.tile([P, NB], BF, tag="tmp_r")
        nc.scalar.activation(tmp_r[:], ps_r[:], AF.Identity,
                             bias=br_sb[:, jc:jc + 1])
        tmp_l = pp.tile([P, NB], BF, tag="tmp_l")
        nc.scalar.activation(tmp_l[:], ps_l[:], AF.Identity,
                             bias=bl_sb[:, jc:jc + 1])

        it = ip.tile([P, NB], BF, tag="inter", bufs=32)
        nc.vector.tensor_mul(it[:], tmp_r[:], tmp_l[:])
        inter.append(it)
        if make_fp8:
            if jc % 2 == 0:
                cur8 = i8p.tile([P, 2 * NB], E4, tag="it8", bufs=16)
                it8s.append(cur8)
            # fp8 copy for the DoubleRow eigen (second DVE mul, fp8 out)
            nc.vector.tensor_mul(cur8[:, (jc % 2) * NB:(jc % 2 + 1) * NB],
                                 tmp_r[:], tmp_l[:])
    for f in fillers:
        f()
    return inter, it8s


def _emit_row_chain(nc, pools, st_mu, st_sq, tag):
    """rstd / -mu*rstd row pair from the PSUM mean/sq-mean rows.

    rstd = sqrt(1/(var+eps)): the reciprocal runs on DVE right after the
    var ops (no engine hop) and the Sqrt writes the bf16 row directly.
    """
    sb, cst = pools["sb"], pools["const"]
    mu = sb.tile([1, NB], F32, tag=f"mu{tag}", bufs=2)
    nc.scalar.copy(mu[:], st_mu)
    var = sb.tile([1, NB], F32, tag=f"var{tag}", bufs=2)
    nc.vector.scalar_tensor_tensor(var[:], mu[:], -1.0, st_mu,
                                   op0=ALU.mult, op1=ALU.mult)
    var2 = sb.tile([1, NB], F32, tag=f"va{tag}", bufs=2)
    nc.vector.scalar_tensor_tensor(var2[:], var[:], LN_EPS, st_sq,
                                   op0=ALU.add, op1=ALU.add)
    rv = sb.tile([1, NB], F32, tag=f"rv{tag}", bufs=2)
    nc.vector.reciprocal_approx_fast(out=rv[:], in_=var2[:])
    row = sb.tile([1, 2 * NB], BF, tag=f"row{tag}", bufs=3)
    nc.scalar.activation(row[:, 0:NB], rv[:], AF.Sqrt)
    nc.vector.scalar_tensor_tensor(row[:, NB:2 * NB], mu[:], -1.0,
                                   row[:, 0:NB], op0=ALU.mult, op1=ALU.mult)
    return row


def _emit_eigen(nc, pools, dram, blk, inter, acts):
    """Eigen projection + residual + LN statistics for one batch tile
    (block 1, bf16). Stats accumulate on DVE running sums; the tail
    (two matvecs + row chain) is emitted inline.
    """
    sb, wep_p, pse, pst, cst = (
        pools["sb"], pools["wep"], pools["ps_e"], pools["ps_st"],
        pools["const"],
    )
    be_sb = cst[f"be{blk}"]
    inv_d = cst["inv_d"]       # [128, 1] bf16 filled with 1/D
    st = pst.tile([64, NB], F32, tag="st")
    mss = [sb.tile([P, NB], BF, tag=f"ms1{g}", bufs=1, name=f"ms1{g}")
           for g in range(EGRP - 1)]
    qss = [sb.tile([P, NB], BF, tag=f"qs1{g}", bufs=1, name=f"qs1{g}")
           for g in range(EGRP - 1)]
    hpre = []
    last_sqs = []

    for g in range(EGRP):
        if g < EGRP - 1:
            ms, qs = mss[g], qss[g]
        ps_es = [pse.tile([P, NB], F32, tag="eig", name=f"eig{i}")
                 for i in range(EGS)]
        for jc in range(JC):
            we_t = wep_p.tile([P, EGS * P], BF, tag="wep")
            nc.sync.dma_start(out=we_t[:], in_=dram[f"we{blk}"][g, jc])
            for di in range(EGS):
                nc.tensor.matmul(
                    ps_es[di][:], we_t[:, di * P:(di + 1) * P], inter[jc][:],
                    start=(jc == 0), stop=(jc == JC - 1),
                )
        # evictions first (unblock hpre consumers), running sums after;
        # one sum pair per group so the last group's chain is short
        for di in range(EGS):
            do = g * EGS + di
            hp = sb.tile([P, NB], BF, tag="hpre", bufs=9)
            nc.vector.scalar_tensor_tensor(hp[:], ps_es[di][:],
                                           be_sb[:, do:do + 1], acts[do][:],
                                           op0=ALU.add, op1=ALU.add)
            hpre.append(hp)
        if g < EGRP - 1:
            for di in range(EGS):
                do = g * EGS + di
                hp = hpre[do]
                if di == 0:
                    nc.vector.scalar_tensor_tensor(ms[:], hp[:], 0.0, hp[:],
                                                   op0=ALU.mult, op1=ALU.add)
                    nc.scalar.activation(qs[:], hp[:], AF.Square)
                else:
                    nc.vector.tensor_add(ms[:], ms[:], hp[:])
                    sq = sb.tile([P, NB], BF, tag="sq", bufs=2)
                    nc.scalar.activation(sq[:], hp[:], AF.Square)
                    nc.vector.tensor_add(qs[:], qs[:], sq[:])
        else:
            # last group: no serial add chain; squares only, the stats
            # matvecs accumulate the raw hp/sq tiles directly
            for di in range(EGS):
                do = g * EGS + di
                sq = sb.tile([P, NB], BF, tag="sqd", bufs=5,
                             name=f"sqd{di}")
                nc.scalar.activation(sq[:], hpre[do][:], AF.Square)
                last_sqs.append(sq)

    box = {}

    def t_mean():
        srcs = [mss[g] for g in range(EGRP - 1)] + hpre[-EGS:]
        for i, s in enumerate(srcs):
            nc.tensor.matmul(st[0:1, :], inv_d[:, 0:1], s[:],
                             start=(i == 0), stop=(i == len(srcs) - 1))

    def t_sq():
        srcs = [qss[g] for g in range(EGRP - 1)] + last_sqs
        for i, s in enumerate(srcs):
            nc.tensor.matmul(st[32:33, :], inv_d[:, 0:1], s[:],
                             start=(i == 0), stop=(i == len(srcs) - 1))

    def t_rowbcast():
        row = _emit_row_chain(nc, pools, st[0:1, :], st[32:33, :], "")
        box["a"], box["c"] = _emit_ln_bcast_bf(nc, pools, row)

    return hpre, box, [t_mean, t_sq, t_rowbcast]



def _emit_rl2_commute(nc, pools, dram, hpre1, box, fillers,
                      late_fillers=None, late_start=8):
    """Block-2 r/l projections consuming the pre-LN residual hpre1 with
    block-1's LN row scalars applied on the (DVE) eviction path:
      r2 = a1 .* psum + c1 .* v~r + kr2
    v~r = (wr2 diag(g1)) @ 1 and kr2 = wr2 @ b1 + br2 are host-folded.
    fillers: block-1's deferred stats matvecs + row/bcast chain, emitted
    between jc=0's matmuls and its eviction so the PE stream never waits
    on them (the scheduler slots them in when their inputs land).
    """
    wp, ip, i8p, pp, psr, cst = (
        pools["wp"], pools["ip"], pools["i8p"], pools["pp"], pools["ps_rl"],
        pools["const"],
    )
    vr_sb, vl_sb = cst["vr2"], cst["vl2"]
    kr_sb, kl_sb = cst["kr2"], cst["kl2"]

    inter = []
    it8s = []
    cur8 = None
    raw0 = None
    for jc in range(JC):
        wr_t = wp.tile([P, D], BF, tag="wrl")
        nc.sync.dma_start(out=wr_t[:], in_=dram["wr2"][jc])
        wl_t = wp.tile([P, D], BF, tag="wrl")
        nc.sync.dma_start(out=wl_t[:], in_=dram["wl2"][jc])

        ps_r = psr.tile([P, NB], F32, tag="rl")
        ps_l = psr.tile([P, NB], F32, tag="rl")
        for dc in range(DC):
            nc.tensor.matmul(
                ps_r[:], wr_t[:, dc * P:(dc + 1) * P], hpre1[dc][:],
                start=(dc == 0), stop=(dc == DC - 1),
            )
            nc.tensor.matmul(
                ps_l[:], wl_t[:, dc * P:(dc + 1) * P], hpre1[dc][:],
                start=(dc == 0), stop=(dc == DC - 1),
            )
        if jc == 0:
            for f in fillers:
                f()
        if late_fillers and jc >= late_start:
            late_fillers.pop(0)()
        a_bf, c_bf = box["a"], box["c"]
        _emit_rl2_evict(nc, pools, ps_r[:], ps_l[:], jc, a_bf, c_bf,
                        inter, it8s)
    return inter, it8s


def _emit_rl2_evict(nc, pools, src_r, src_l, jc, a_bf, c_bf, inter, it8s):
    """LN-fixup eviction + interaction for one jc chunk of block-2's r/l.
    src_r/src_l may be PSUM banks or raw bf16 SBUF tiles."""
    ip, i8p, pp, cst = (pools["ip"], pools["i8p"], pools["pp"],
                        pools["const"])
    vr_sb, vl_sb = cst["vr2"], cst["vl2"]
    kr_sb, kl_sb = cst["kr2"], cst["kl2"]
    u_r = pp.tile([P, NB], BF, tag="u")
    nc.vector.tensor_mul(u_r[:], src_r, a_bf[:])
    tmp_r = pp.tile([P, NB], BF, tag="tmp_r")
    nc.vector.scalar_tensor_tensor(tmp_r[:], c_bf[:], vr_sb[:, jc:jc + 1],
                                   u_r[:], op0=ALU.mult, op1=ALU.add)
    u_l = pp.tile([P, NB], BF, tag="w")
    nc.vector.tensor_mul(u_l[:], src_l, a_bf[:])
    tmp_l = pp.tile([P, NB], BF, tag="tmp_l")
    nc.vector.scalar_tensor_tensor(tmp_l[:], c_bf[:], vl_sb[:, jc:jc + 1],
                                   u_l[:], op0=ALU.mult, op1=ALU.add)
    tmp_lk = pp.tile([P, NB], BF, tag="tmp_e2", bufs=2)
    nc.scalar.activation(tmp_lk[:], tmp_l[:], AF.Identity,
                         bias=kl_sb[:, jc:jc + 1])

    # inter = (tmp_r + kr) * (tmp_l + kl); fp8 copy first (the
    # DoubleRow eigen consumes it sooner than the head reads bf16)
    if jc % 2 == 0:
        cur8 = i8p.tile([P, 2 * NB], E4, tag="it8", bufs=16,
                        name=f"it8_{jc}")
        it8s.append(cur8)
    cur8 = it8s[-1]
    nc.vector.scalar_tensor_tensor(
        cur8[:, (jc % 2) * NB:(jc % 2 + 1) * NB], tmp_r[:],
        kr_sb[:, jc:jc + 1], tmp_lk[:], op0=ALU.add, op1=ALU.mult)
    it = ip.tile([P, NB], BF, tag="inter", bufs=32, name=f"it_{jc}")
    nc.vector.scalar_tensor_tensor(it[:], tmp_r[:], kr_sb[:, jc:jc + 1],
                                   tmp_lk[:], op0=ALU.add, op1=ALU.mult)
    inter.append(it)


def _emit_eigen2(nc, pools, dram, inter, it8s, h1, final=False):
    """Block-2 eigen in fp8 DoubleRow (feeds LN stats only) + exact head
    accumulation.

    st bank rows: 0:OUT head accumulator, 32:33 mean, 64:65 sq-mean.
    Returns (st, row_box, tail); tail = two stats matvecs + row chain,
    deferred into the next tile's r/l stream (or interleaved into the
    head matmul stream when final=True).
    """
    sb, wep_p, pp, pse, pst, cst = (
        pools["sb"], pools["wep"], pools["pp"], pools["ps_e"],
        pools["ps_st"], pools["const"],
    )
    be_sb = cst["be2"]
    inv_d = cst["inv_d"]
    st = pst.tile([P, NB], F32, tag="st")
    ng = EGRP - 1 if final else EGRP
    mss = [sb.tile([P, NB], BF, tag=f"ms2{g}", bufs=1, name=f"ms2{g}")
           for g in range(ng)]
    qss = [sb.tile([P, NB], BF, tag=f"qs2{g}", bufs=1, name=f"qs2{g}")
           for g in range(ng)]
    last_hps = []
    last_sqs = []

    for g in range(EGRP):
        direct = final and g == EGRP - 1
        if not direct:
            ms, qs = mss[g], qss[g]
        ps_es = [pse.tile([P, NB], F32, tag="eig", name=f"e2g{i}")
                 for i in range(EGS)]
        for kp in range(KP):
            we_t = wep_p.tile([P, EGS * 2 * P], E4, tag="wep8", bufs=8)
            nc.sync.dma_start(out=we_t[:], in_=dram["we2q8"][g, kp])
            rhs = it8s[kp][:].rearrange("p (two n) -> p two n", two=2)
            for di in range(EGS):
                lhsT = we_t[:, di * 2 * P:(di + 1) * 2 * P].rearrange(
                    "p (two m) -> p two m", two=2)
                nc.tensor.matmul(ps_es[di][:], lhsT, rhs,
                                 start=(kp == 0), stop=(kp == KP - 1),
                                 perf_mode=PM.DoubleRow)
        for di in range(EGS):
            do = g * EGS + di
            # dequant eviction: hp = (psum/WS_E2 + be2) + h1
            t8 = pp.tile([P, NB], BF, tag="tmp_e2", bufs=2)
            nc.scalar.activation(t8[:], ps_es[di][:], AF.Identity,
                                 bias=be_sb[:, do:do + 1], scale=1.0 / WS_E2)
            hp = pp.tile([P, NB], BF, tag="hp2", bufs=5)
            nc.vector.tensor_add(hp[:], t8[:], h1[do][:])
            if direct:
                # final tile: no serial add chain; the stats matvecs
                # accumulate hp/sq tiles directly
                last_hps.append(hp)
                sq = pp.tile([P, NB], BF, tag="sq2", bufs=5)
                nc.scalar.activation(sq[:], hp[:], AF.Square)
                last_sqs.append(sq)
            elif di == 0:
                nc.vector.scalar_tensor_tensor(ms[:], hp[:], 0.0, hp[:],
                                               op0=ALU.mult, op1=ALU.add)
                nc.scalar.activation(qs[:], hp[:], AF.Square)
            else:
                nc.vector.tensor_add(ms[:], ms[:], hp[:])
                sq = pp.tile([P, NB], BF, tag="sq2", bufs=5)
                nc.scalar.activation(sq[:], hp[:], AF.Square)
                nc.vector.tensor_add(qs[:], qs[:], sq[:])

    row_box = {}

    def t_mean():
        srcs = list(mss) + last_hps
        for i, s in enumerate(srcs):
            nc.tensor.matmul(st[32:33, :], inv_d[:, 0:1], s[:],
                             start=(i == 0), stop=(i == len(srcs) - 1))

    def t_sq():
        srcs = list(qss) + last_sqs
        for i, s in enumerate(srcs):
            nc.tensor.matmul(st[64:65, :], inv_d[:, 0:1], s[:],
                             start=(i == 0), stop=(i == len(srcs) - 1))

    def t_row():
        row_box["row"] = _emit_row_chain(nc, pools, st[32:33, :],
                                         st[64:65, :], "2")

    tail = [t_mean, t_sq, t_row]

    # exact head accumulation into rows 0:OUT of the same bank:
    #   hd = wf_g2 @ h1 + Wfe2 @ inter2   (all inputs long-ready)
    for dc in range(DC):
        nc.tensor.matmul(st[0:OUT, :], cst["wf"][:, dc * OUT:(dc + 1) * OUT],
                         h1[dc][:], start=(dc == 0), stop=False)
    for jc in range(JC):
        if final and tail and jc in (2, 4, 6):
            tail.pop(0)()
        nc.tensor.matmul(st[0:OUT, :], cst["wfe"][:, jc * OUT:(jc + 1) * OUT],
                         inter[jc][:], start=False, stop=(jc == JC - 1))
    if final:
        for f in tail:
            f()
        tail = []
    return st, row_box, tail


def _emit_ln_bcast_bf(nc, pools, row):
    """Broadcast [a | c] across partitions (two K=1 bf16 matmuls) and
    evict to bf16 SBUF immediately so the PSUM banks free early and the
    apply runs at bf16 DVE rate."""
    sb, pse, cst = pools["sb"], pools["ps_e"], pools["const"]
    ones_r = cst["ones_r"]
    a_ps = pse.tile([P, NB], F32, tag="eig", name="a_b")
    nc.tensor.matmul(a_ps[:], ones_r[:, :], row[:, 0:NB], start=True,
                     stop=True)
    c_ps = pse.tile([P, NB], F32, tag="eig", name="c_b")
    nc.tensor.matmul(c_ps[:], ones_r[:, :], row[:, NB:2 * NB],
                     start=True, stop=True)
    a_bf = sb.tile([P, NB], BF, tag="abf", bufs=2)
    nc.scalar.copy(a_bf[:], a_ps[:])
    c_bf = sb.tile([P, NB], BF, tag="cbf", bufs=2)
    nc.scalar.copy(c_bf[:], c_ps[:])
    return a_bf, c_bf


def _make_ln_apply(nc, pools, blk, hpre, box, outs):
    """Per-chunk LN-apply closures (2 DVE + 1 ACT each), interleaved into
    block-2's r/l stream so h1 is materialized while the PE streams."""
    sb, pp, cst = pools["sb"], pools["pp"], pools["const"]
    g_sb = cst[f"g{blk}"]
    bb_sb = cst[f"bb{blk}"]

    def one(do):
        def emit():
            u = pp.tile([P, NB], BF, tag="u")
            nc.vector.tensor_mul(u[:], hpre[do][:], box["a"][:])
            w = pp.tile([P, NB], BF, tag="w")
            nc.vector.tensor_add(w[:], u[:], box["c"][:])
            ho = sb.tile([P, NB], BF, tag=f"h{blk}", bufs=9)
            nc.scalar.activation(ho[:], w[:], AF.Identity,
                                 bias=bb_sb[:, do:do + 1],
                                 scale=g_sb[:, do:do + 1])
            outs.append(ho)
        return emit

    return [one(do) for do in range(DC)]


def build_program(bc=BC):
    """Build the per-core SPMD program. bc = rows per core."""
    nt = bc // NB
    nc = bacc.Bacc("TRN2", target_bir_lowering=False)

    dram = {
        "xT": nc.dram_tensor("xT", [D, bc], BF, kind="ExternalInput"),
        # wf is pre-folded with the block-2 LN gain g2 (host side)
        "wf": nc.dram_tensor("wf", [P, DC * OUT], BF, kind="ExternalInput"),
        # wfe = (wf_g2 @ we2) panels, contraction over the bilinear dim
        "wfe": nc.dram_tensor("wfe", [P, JC * OUT], BF, kind="ExternalInput"),
        "sf": nc.dram_tensor("sf", [OUT, 1], F32, kind="ExternalInput"),
        "tf": nc.dram_tensor("tf", [OUT, 1], F32, kind="ExternalInput"),
        "behead": nc.dram_tensor("behead", [OUT, 1], F32,
                                 kind="ExternalInput"),
        "outT": nc.dram_tensor("outT", [OUT, bc], F32, kind="ExternalOutput"),
        # block-2 eigen weights, fp8e4, pre-scaled by WS_E2, DoubleRow
        # k-pair layout [g, kp, p_j, (di, two, p_d)]
        "we2q8": nc.dram_tensor("we2q8", [EGRP, KP, P, EGS * 2 * P], E4,
                                kind="ExternalInput"),
    }
    for blk in (1, 2):
        dram[f"wr{blk}"] = nc.dram_tensor(f"wr{blk}", [JC, P, D], BF,
                                          kind="ExternalInput")
        dram[f"wl{blk}"] = nc.dram_tensor(f"wl{blk}", [JC, P, D], BF,
                                          kind="ExternalInput")
        dram[f"be{blk}"] = nc.dram_tensor(f"be{blk}", [P, DC], F32,
                                          kind="ExternalInput")
    for nm in ("br1", "bl1", "vr2", "vl2", "kr2", "kl2"):
        dram[nm] = nc.dram_tensor(nm, [P, JC], F32, kind="ExternalInput")
    for nm in ("g1", "bb1"):
        dram[nm] = nc.dram_tensor(nm, [P, DC], F32, kind="ExternalInput")
    if True:
        pass
    dram["we1"] = nc.dram_tensor("we1", [EGRP, JC, P, EGS * P], BF,
                                 kind="ExternalInput")

    with tile.TileContext(nc) as tc:
        with (
            tc.tile_pool(name="sb", bufs=2) as sb,
            tc.tile_pool(name="wp", bufs=6) as wp,
            tc.tile_pool(name="wep", bufs=12) as wep_p,
            tc.tile_pool(name="ip", bufs=32) as ip,
            tc.tile_pool(name="i8p", bufs=16) as i8p,
            tc.tile_pool(name="pp", bufs=3) as pp,
            tc.tile_pool(name="const", bufs=1) as cstp,
            tc.tile_pool(name="ps_rl", bufs=3, space="PSUM") as ps_rl,
            tc.tile_pool(name="ps_e", bufs=4, space="PSUM") as ps_e,
            tc.tile_pool(name="ps_st", bufs=1, space="PSUM") as ps_st,
        ):
            # warmup first: memset-fed throwaway matmuls start the PE before
            # any DMA lands, lifting the HAM clock gate to 8/8 early
            wm_l = cstp.tile([P, P], BF, tag="wm_l", name="wm_l")
            nc.vector.memset(wm_l[:], 0.0)
            wm_r = cstp.tile([P, NB], BF, tag="wm_r", name="wm_r")
            nc.vector.memset(wm_r[:], 0.0)
            for i in range(16):
                wps = ps_rl.tile([P, NB], F32, tag="rl", name=f"warm{i}")
                nc.tensor.matmul(wps[:], wm_l[:], wm_r[:],
                                 start=True, stop=True)

            cst = {}
            const_names = [("br1", JC), ("bl1", JC), ("vr2", JC),
                           ("vl2", JC), ("kr2", JC), ("kl2", JC),
                           ("be1", DC), ("be2", DC), ("g1", DC), ("bb1", DC)]
            for nm, cols in const_names:
                cst[nm] = cstp.tile([P, cols], F32, tag=nm, name=nm)
                nc.gpsimd.dma_start(out=cst[nm][:], in_=dram[nm][:])
            cst["inv_d"] = cstp.tile([P, 1], BF, tag="inv_d", name="inv_d")
            nc.vector.memset(cst["inv_d"][:], 1.0 / D)
            cst["ones_r"] = cstp.tile([1, P], BF, tag="ones_r", name="ones_r")
            nc.vector.memset(cst["ones_r"][:], 1.0)
            cst["eps"] = cstp.tile([1, 1], F32, tag="eps", name="eps")
            nc.vector.memset(cst["eps"][:], LN_EPS)
            cst["wf"] = cstp.tile([P, DC * OUT], BF, tag="wf", name="wf_sb")
            nc.gpsimd.dma_start(out=cst["wf"][:], in_=dram["wf"][:])
            cst["wfe"] = cstp.tile([P, JC * OUT], BF, tag="wfe", name="wfe_sb")
            nc.gpsimd.dma_start(out=cst["wfe"][:], in_=dram["wfe"][:])
            for nm in ("sf", "tf", "behead"):
                cst[nm] = cstp.tile([OUT, 1], F32, tag=nm, name=f"{nm}_sb")
                nc.gpsimd.dma_start(out=cst[nm][:], in_=dram[nm][:])

            pools = {
                "sb": sb, "wp": wp, "wep": wep_p, "ip": ip, "i8p": i8p,
                "pp": pp, "const": cst, "ps_rl": ps_rl, "ps_e": ps_e,
                "ps_st": ps_st,
            }
            ones_r = cst["ones_r"]

            def emit_head_apply(st, row, t):
                """out = a2 .* (hd + behead) + sf (x) c2 + tf, from the
                head accumulator in st rows 0:OUT."""
                a_ps = ps_e.tile([P, NB], F32, tag="eig", name="ha_b")
                nc.tensor.matmul(a_ps[0:OUT, :], ones_r[:, 0:OUT],
                                 row[:, 0:NB], start=True, stop=True)
                c_ps = ps_e.tile([P, NB], F32, tag="eig", name="hc_b")
                nc.tensor.matmul(c_ps[0:OUT, :], ones_r[:, 0:OUT],
                                 row[:, NB:2 * NB], start=True, stop=True)
                hd2 = sb.tile([OUT, NB], F32, tag="hd2", bufs=1)
                nc.scalar.activation(hd2[:], st[0:OUT, :], AF.Identity,
                                     bias=cst["behead"][:])
                a_sb = sb.tile([OUT, NB], F32, tag="hab", bufs=1)
                nc.scalar.copy(a_sb[:], a_ps[0:OUT, :])
                u = sb.tile([OUT, NB], F32, tag="hu", bufs=1)
                nc.vector.tensor_mul(u[:], hd2[:], a_sb[:])
                v = sb.tile([OUT, NB], F32, tag="hv", bufs=1)
                nc.vector.scalar_tensor_tensor(v[:], c_ps[0:OUT, :],
                                               cst["sf"][:], u[:],
                                               op0=ALU.mult, op1=ALU.add)
                out_sb = sb.tile([OUT, NB], F32, tag="osb", bufs=2)
                nc.scalar.activation(out_sb[:], v[:], AF.Identity,
                                     bias=cst["tf"][:])
                nc.gpsimd.dma_start(out=dram["outT"][:, t * NB:(t + 1) * NB],
                                    in_=out_sb[:])

            # pending = (st2, row_box, t, tail) for the tile whose block-2
            # stats matvecs + row chain + head application are deferred
            # into the next tile's block-1 r/l stream (fillers). Running
            # them early also frees the shared stats/head PSUM bank before
            # the next tile's block-1 eigen needs it.
            pending = None
            for t in range(nt):
                x_bf = []
                for dc in range(DC):
                    xt = sb.tile([P, NB], BF, tag="xbf", bufs=9)
                    xq = nc.scalar if t == 0 else nc.sync
                    xq.dma_start(
                        out=xt[:],
                        in_=dram["xT"][dc * P:(dc + 1) * P,
                                       t * NB:(t + 1) * NB],
                    )
                    x_bf.append(xt)

                if pending is not None:
                    st_prev, row2_box, t_prev, tail_prev = pending
                    prev_tail = list(tail_prev)
                    prev_tail.append(
                        lambda s=st_prev, rb=row2_box, tp=t_prev:
                        emit_head_apply(s, rb["row"], tp))
                else:
                    prev_tail = []
                inter1, _ = _emit_rl(nc, pools, dram, 1, x_bf,
                                     fillers=prev_tail)
                hpre1, box1, tail1 = _emit_eigen(nc, pools, dram, 1, inter1,
                                                 x_bf)
                h1 = []
                apply_fs = _make_ln_apply(nc, pools, 1, hpre1, box1, h1)
                final = (t == nt - 1)
                inter2, it8s = _emit_rl2_commute(nc, pools, dram, hpre1,
                                                 box1, tail1,
                                                 late_fillers=apply_fs,
                                                 late_start=2 if final
                                                 else 8)
                st2, row2_box, tail2 = _emit_eigen2(nc, pools, dram, inter2,
                                                    it8s, h1, final=final)
                pending = (st2, row2_box, t, tail2)

            # final tile: its tail was interleaved into the head stream
            st_prev, row2_box, t_prev, _ = pending
            emit_head_apply(st_prev, row2_box["row"], t_prev)
    nc.compile()
    return nc


def _bf(a):
    return np.ascontiguousarray(a.astype(ml_dtypes.bfloat16))


def prep_inputs(inputs, bc=BC, ncores=NCORES):
    """Host-side shard + transpose + bf16/fp8 conversion. Returns in_maps."""
    f = {k: np.asarray(v, dtype=np.float32) for k, v in inputs.items()}

    shared = {}
    for side in ("r", "l"):
        w = f[f"w{side}1"].reshape(HR, D)                  # [j, d]
        panel = w.reshape(JC, P, DC, P).transpose(0, 3, 2, 1)
        shared[f"w{side}1"] = _bf(panel.reshape(JC, P, D))
        shared[f"b{side}1"] = np.ascontiguousarray(
            f[f"b{side}1"].reshape(JC, P).T)                # [128, 32]
    # block-2 r/l: g1-folded panels + LN-commute fixup vectors
    g1_64 = f["g1"].astype(np.float64)
    b1_64 = f["b1"].astype(np.float64)
    for side in ("r", "l"):
        w64 = f[f"w{side}2"].reshape(HR, D).astype(np.float64)
        wg = w64 * g1_64[None, :]                           # W~ = W diag(g1)
        panel = wg.astype(np.float32).reshape(JC, P, DC, P).transpose(
            0, 3, 2, 1)
        shared[f"w{side}2"] = _bf(panel.reshape(JC, P, D))
        v = wg.sum(axis=1)                                  # v~ = W~ @ 1
        shared[f"v{side}2"] = np.ascontiguousarray(
            v.astype(np.float32).reshape(JC, P).T)          # [128, 32]
        k = w64 @ b1_64 + f[f"b{side}2"].reshape(HR).astype(np.float64)
        shared[f"k{side}2"] = np.ascontiguousarray(
            k.astype(np.float32).reshape(JC, P).T)          # [128, 32]
    for blk in (1, 2):
        shared[f"be{blk}"] = np.ascontiguousarray(
            f[f"be{blk}"].reshape(DC, P).T)                 # [128, 8]
    shared["g1"] = np.ascontiguousarray(f["g1"].reshape(DC, P).T)
    shared["bb1"] = np.ascontiguousarray(f["b1"].reshape(DC, P).T)

    # block-1 eigen: bf16 panels [g, jc, p_j, (di, p_d)]
    weT = f["we1"].T                                        # [j, d_out]
    panel = weT.reshape(JC, P, EGRP, EGS * P).transpose(2, 0, 1, 3)
    shared["we1"] = _bf(panel)                              # [g, jc, p, 512]

    # block-2 eigen: fp8e4 DoubleRow panels [g, kp, p_j, (di, two, p_d)],
    # pre-scaled so weight values sit in fp8's normal range
    weT2 = f["we2"].T                                       # [4096, 1024]
    pan8 = weT2.reshape(KP, 2, P, EGRP, EGS, P).transpose(3, 0, 2, 4, 1, 5)
    pan8 = np.clip(pan8 * WS_E2, -240.0, 240.0)
    shared["we2q8"] = np.ascontiguousarray(
        pan8.reshape(EGRP, KP, P, EGS * 2 * P).astype(ml_dtypes.float8_e4m3))

    # head folding (block-2 LN never applied as tensors):
    #   out = a2 .* (wf_g2 @ h1 + Wfe2 @ inter2 + behead) + sf (x) c2 + tf
    wf64 = f["wf"].astype(np.float64)
    g2_64 = f["g2"].astype(np.float64)
    we2_64 = f["we2"].astype(np.float64)
    wf_g2 = wf64 * g2_64[None, :]                           # [OUT, D]
    shared["wf"] = _bf(wf_g2.astype(np.float32).T.reshape(DC, P, OUT)
                       .transpose(1, 0, 2).reshape(P, DC * OUT))
    wfe2 = wf_g2 @ we2_64                                   # [OUT, HR]
    shared["wfe"] = _bf(wfe2.astype(np.float32).T.reshape(JC, P, OUT)
                        .transpose(1, 0, 2).reshape(P, JC * OUT))
    shared["behead"] = np.ascontiguousarray(
        (wf_g2 @ f["be2"].astype(np.float64)).reshape(OUT, 1)
        .astype(np.float32))
    shared["sf"] = np.ascontiguousarray(
        wf_g2.sum(axis=1).reshape(OUT, 1).astype(np.float32))
    shared["tf"] = np.ascontiguousarray(
        (wf64 @ f["b2"].astype(np.float64) + f["bf"]).reshape(OUT, 1)
        .astype(np.float32))

    x = f["x"]
    in_maps = []
    for c in range(ncores):
        m = dict(shared)
        m["xT"] = _bf(x[c * bc:(c + 1) * bc].T)             # [1024, bc]
        in_maps.append(m)
    return in_maps


_PROGRAM_CACHE = {}


def get_program(bc=BC):
    if bc not in _PROGRAM_CACHE:
        _PROGRAM_CACHE[bc] = build_program(bc)
    return _PROGRAM_CACHE[bc]


def kernel(**inputs):
    nc = get_program(BC)
    in_maps = prep_inputs(inputs, BC, NCORES)
    res = run_bass_kernel_spmd(nc, in_maps, core_ids=list(range(NCORES)))
    out = np.concatenate([res.results[c]["outT"] for c in range(NCORES)],
                         axis=1).T
    return np.ascontiguousarray(out.astype(np.float32))


if __name__ == "__main__":
    raise SystemExit("import kernel and call kernel(**inputs); see test.py")
